# revision 49
# baseline (speedup 1.0000x reference)
"""SMPL (shape blend + pose blend + LBS skinning) Bass kernel for 8 TRN2 NeuronCores.

Data-parallel over batch: B=1024 -> 128 per core. All SMPL buffers replicated.

Measured HW model this kernel is built around (NTFF traces on these cores):
  - PE runs at 1.2 GHz here (no HAM ramp observed), 1 psum column/cycle,
    out <= 512 fp32 cols per matmul (1 PSUM bank), ~300ns fixed cost per
    matmul + ~420ns LDWEIGHTS+gap, partially hidden by the queue.
  - fp8 DoubleRow streams 2 packed columns/cycle -> same out-column rate as
    fp16, but doubles K capacity per pass: used for the K=230 vp matmul
    (one mm instead of two per c-plane).
  - ACTIVATE (ScalarE) is 1x, ~(N+352)/1.2 ns; DVE fp16 tensor_tensor is 2x;
    a DVE op overlapping any GpSimd op drops to ~0.5x (shared SBUF port), so
    GpSimd is left idle on purpose.

Numerics: vp in fp8e4 DoubleRow with power-of-2 row scaling and a hi/lo
compensation split for v_template; skinning matmul T in fp16.
Measured rel err 7.7e-3 vs the 2e-2 gate.
  vp (K=220 packed into 110 rows x 2 panels):
      rows = [beta/16 x shapedirs*16 | lrot/16 x posedirs*16 |
              1 x tmpl_hi | 1/16 x tmpl_lo*16 | pad]
  (110-row DMA tiles are ~2x faster per byte than 115-row ones - the DMA
  rate is sharply sensitive to partition-row count; dirs loads are issued
  at chunk-PAIR granularity, 6KB per partition row.)

Phase structure (per core):
  prologue: ACT-table warmup at t=0; Rodrigues (V+S) -> scaled coeff ->
            fp32 transposes -> coeffT8 (fp8); J matmul; FK on V
  P1 loop (overlaps FK on V): 3 DR matmuls/chunk -> 1-bank vp psum tiles ->
            per-plane S copies into persistent vp_sb [128,3,7168] fp16. The
            12 gat transposes are emitted mid-P1 (TSPLIT=12) so the PE
            reaches them right as FK finishes; remaining P1 chunks keep the
            PE queue fed.
  P2 loop:  12 fp16 matmuls/chunk into 3-bank psum n-groups (bufs=2) +
            pt3 group; S evicts all 4 groups to fp16 (incl pt3 - frees the
            psum slot fast and keeps V in 2x mode); V: 3 broadcast muls +
            3-add chain (never in-place: dst==src DVE ops run 4x slower).
Last chunk runs at its true 234-col width. Output [14, 128, 3, 512] fp16
chunk-major; host reassembles to [1024, 6890, 3] fp32.
"""

import sys
import numpy as np
import ml_dtypes

for _p in ("/opt/trn_rl_repo",):
    if _p not in sys.path:
        sys.path.append(_p)

import concourse.bass as bass
import concourse.tile as tile
import concourse.mybir as mybir
from concourse import bacc
from concourse.bass_utils import run_bass_kernel_spmd
from concourse.alu_op_type import AluOpType

F32 = mybir.dt.float32
F16 = mybir.dt.float16
F8 = mybir.dt.float8e4
F8NP = ml_dtypes.float8_e4m3
DR = mybir.MatmulPerfMode.DoubleRow

N_CORES = 8
B = 1024
B_LOC = B // N_CORES  # 128
NV = 6890
NVP = 7168            # padded to 14*512
NCH = 14
CH = 512
NJ = 24
NP = 207

KVP = 220             # vp contraction (padded even): 10+207+2+1
KVH = KVP // 2        # 110
KT = 64               # T contraction rows (x2 panels = 128 logical);
                      # blocks at 32-aligned bases: hi @0, lo8 @32, hi16 @0/p1

# FK level groups: (child_start, n_children, parent_start, parent_broadcast)
FK_GROUPS = [
    (1, 3, 0, True),
    (4, 3, 1, False),
    (7, 3, 4, False),
    (10, 3, 7, False),
    (13, 2, 9, True),
    (15, 3, 12, False),
    (18, 2, 16, False),
    (20, 2, 18, False),
    (22, 2, 20, False),
]

CFG = {
    "compute": "fp8",    # "fp8" | "fp16" (legacy)
    "ch": 512,
    "out16": True,
    "vp_fp8": False,
    "debug": False,
    "trace": False,
}

_CACHE = {}


def _rodrigues_and_coeff(nc, tc, constp, statep, scrp, pose_sb, beta_sb):
    """Rodrigues rotation build (fp32) + scaled coeff [128, KVP] fp32.

    coeff rows: 0..9 beta/16, 10..216 lrot/16, 217 = 1.0, 218 = 1/16,
    219 = 0.
    Returns (r9, coeff, beta_sb)."""
    V = nc.vector
    S = nc.scalar

    # dummy activation at t=0: pulls the Sqrt ACT-table load off the
    # Rodrigues critical path (it overlaps the pose DMA instead). Only the
    # Sqrt set is warmed: warming Sin too would thrash (the table RAM holds
    # one set at a time, so Sqrt would reload on the path; measured 4 loads
    # instead of 2).
    warm = scrp.tile([B_LOC, 1], F32, tag="warm")
    V.memset(warm[:, :], 1.0)
    S.activation(warm[:, :], warm[:, :], mybir.ActivationFunctionType.Sqrt)

    sq = scrp.tile([B_LOC, 72], F32, tag="sq")
    V.tensor_mul(sq[:, :], pose_sb[:, :], pose_sb[:, :])
    sq3 = sq[:, :].rearrange("p (j c) -> p c j", c=3)
    th2 = scrp.tile([B_LOC, NJ], F32, tag="th2")
    V.tensor_add(th2[:, :], sq3[:, 0, :], sq3[:, 1, :])
    V.tensor_add(th2[:, :], th2[:, :], sq3[:, 2, :])
    cbias = constp.tile([128, 2], F32)
    V.memset(cbias[:, 0:1], 1e-8)
    V.memset(cbias[:, 1:2], float(np.pi / 2))
    theta = scrp.tile([B_LOC, NJ], F32, tag="theta")
    S.activation(theta[:, :], th2[:, :], mybir.ActivationFunctionType.Sqrt,
                 bias=cbias[0:B_LOC, 0:1])
    invt = scrp.tile([B_LOC, NJ], F32, tag="invt")
    V.reciprocal(invt[:, :], theta[:, :])
    sh = scrp.tile([B_LOC, NJ], F32, tag="sh")
    S.activation(sh[:, :], theta[:, :], mybir.ActivationFunctionType.Sin, scale=0.5)
    chh = scrp.tile([B_LOC, NJ], F32, tag="chh")
    S.activation(chh[:, :], theta[:, :], mybir.ActivationFunctionType.Sin,
                 scale=0.5, bias=cbias[0:B_LOC, 1:2])
    s_t = scrp.tile([B_LOC, NJ], F32, tag="s_t")
    V.scalar_tensor_tensor(s_t[:, :], sh[:, :], 2.0, chh[:, :], AluOpType.mult, AluOpType.mult)
    shsq = scrp.tile([B_LOC, NJ], F32, tag="shsq")
    V.tensor_mul(shsq[:, :], sh[:, :], sh[:, :])
    c_t = scrp.tile([B_LOC, NJ], F32, tag="c_t")
    V.tensor_scalar(c_t[:, :], shsq[:, :], -2.0, 1.0, AluOpType.mult, AluOpType.add)
    omc = scrp.tile([B_LOC, NJ], F32, tag="omc")
    V.tensor_scalar_mul(omc[:, :], shsq[:, :], 2.0)
    ax = scrp.tile([B_LOC, 72], F32, tag="ax")
    ax3 = ax[:, :].rearrange("p (j c) -> p c j", c=3)
    p3 = pose_sb[:, :].rearrange("p (j c) -> p c j", c=3)
    V.tensor_mul(ax3[:, :, :], p3[:, :, :],
                 invt[:, :].unsqueeze(1).broadcast_to([B_LOC, 3, NJ]))
    # batched outer products: [xx yy zz xy xz yz sx sy sz] in 5 ops instead
    # of 15 (each ~180ns of serial V time on the coeffT8 critical chain)
    pr9 = scrp.tile([B_LOC, 9, NJ], F32, tag="pr9")
    prm = scrp.tile([B_LOC, 6, NJ], F32, tag="prm")
    V.tensor_mul(pr9[:, 0:3, :], ax3[:, 0:3, :], ax3[:, 0:3, :])
    V.tensor_mul(pr9[:, 3:5, :],
                 ax3[:, 0:1, :].broadcast_to([B_LOC, 2, NJ]), ax3[:, 1:3, :])
    V.tensor_mul(pr9[:, 5:6, :], ax3[:, 1:2, :], ax3[:, 2:3, :])
    V.tensor_mul(prm[:, :, :], pr9[:, 0:6, :],
                 omc[:, :].unsqueeze(1).broadcast_to([B_LOC, 6, NJ]))
    V.tensor_mul(pr9[:, 6:9, :],
                 s_t[:, :].unsqueeze(1).broadcast_to([B_LOC, 3, NJ]),
                 ax3[:, 0:3, :])
    prods = {"xx": prm[:, 0, :], "yy": prm[:, 1, :], "zz": prm[:, 2, :],
             "xy": prm[:, 3, :], "xz": prm[:, 4, :], "yz": prm[:, 5, :],
             "sx": pr9[:, 6, :], "sy": pr9[:, 7, :], "sz": pr9[:, 8, :]}
    r9 = statep.tile([B_LOC, NJ * 9], F32)
    r9e = r9[:, :].rearrange("p (j e) -> p e j", e=9)
    ENTRIES = [
        ("add", "c", "xx"), ("sub", "xy", "sz"), ("add", "xz", "sy"),
        ("add", "xy", "sz"), ("add", "c", "yy"), ("sub", "yz", "sx"),
        ("sub", "xz", "sy"), ("add", "yz", "sx"), ("add", "c", "zz"),
    ]
    for e, (op, a, b_) in enumerate(ENTRIES):
        ta = c_t[:, :] if a == "c" else prods[a]
        fn = V.tensor_add if op == "add" else V.tensor_sub
        fn(r9e[:, e, :], ta, prods[b_])

    # ---- scaled coeff ----
    coeff = statep.tile([B_LOC, KVP], F32)
    V.tensor_scalar_mul(coeff[:, 0:10], beta_sb[:, :], 1.0 / 16.0)
    # lrot/16 with diag -1/16
    V.tensor_scalar_mul(coeff[:, 10:217], r9[:, 9:216], 1.0 / 16.0)
    lr9 = coeff[:, 10:217].rearrange("p (j e) -> p e j", e=9)
    for e in (0, 4, 8):
        V.tensor_scalar_add(lr9[:, e, :], lr9[:, e, :], -1.0 / 16.0)
    V.memset(coeff[:, 217:218], 1.0)
    V.memset(coeff[:, 218:219], 1.0 / 16.0)
    V.memset(coeff[:, 219:220], 0.0)
    return r9, coeff, beta_sb


def build_program_fp8(cfg):
    key = ("fp8", cfg["debug"])
    if key in _CACHE:
        return _CACHE[key]

    nc = bacc.Bacc("TRN2", target_bir_lowering=False, debug=False)

    pose_d = nc.dram_tensor("pose", [B_LOC, 72], F32, kind="ExternalInput")
    beta_d = nc.dram_tensor("beta", [B_LOC, 10], F32, kind="ExternalInput")
    # chunk-PAIR granularity: 6144B per partition row per DMA (115 x 3KB
    # descriptors measured only ~23GB/s/engine; P1 was DMA-bandwidth bound
    # at 2.85us/chunk)
    dirs8_d = nc.dram_tensor("dirs8", [NCH // 2, KVH, 2, 2, 3, CH], F8,
                             kind="ExternalInput")
    wt16_d = nc.dram_tensor("wt16", [NCH, NJ, CH], F16, kind="ExternalInput")
    js2_d = nc.dram_tensor("js2", [10, 72], F32, kind="ExternalInput")
    jtmpl_d = nc.dram_tensor("jtmpl", [1, 72], F32, kind="ExternalInput")
    ident_d = nc.dram_tensor("ident", [128, 128], F32, kind="ExternalInput")
    out_d = nc.dram_tensor("out", [NCH, B_LOC, 3, CH], F16, kind="ExternalOutput")
    dbg = {}
    if cfg["debug"]:
        dbg["r9"] = nc.dram_tensor("dbg_r9", [B_LOC, 216], F32, kind="ExternalOutput")
        dbg["j"] = nc.dram_tensor("dbg_j", [B_LOC, 72], F32, kind="ExternalOutput")
        dbg["gw"] = nc.dram_tensor("dbg_gw", [B_LOC, 288], F32, kind="ExternalOutput")
        dbg["vp"] = nc.dram_tensor("dbg_vp", [B_LOC, 3, NVP], F16, kind="ExternalOutput")

    with tile.TileContext(nc) as tc:
        with (
            tc.tile_pool(name="const", bufs=1) as constp,
            tc.tile_pool(name="state", bufs=1) as statep,
            tc.tile_pool(name="scr", bufs=1) as scrp,
        ):
            V = nc.vector
            S = nc.scalar
            # pose/beta first on the sync queue: they gate the whole
            # Rodrigues critical chain (ident/js2/jtmpl aren't needed until
            # the transposes ~10us later)
            pose_sb = statep.tile([B_LOC, 72], F32)
            nc.sync.dma_start(pose_sb[:, :], pose_d.ap())
            beta_sb = statep.tile([B_LOC, 10], F32)
            nc.sync.dma_start(beta_sb[:, :], beta_d.ap())
            ident = constp.tile([128, 128], F32)
            nc.sync.dma_start(ident[:, :], ident_d.ap())
            js2 = statep.tile([10, 72], F32)
            nc.sync.dma_start(js2[:, :], js2_d.ap())
            jtmpl = statep.tile([1, 72], F32)
            nc.sync.dma_start(jtmpl[:, :], jtmpl_d.ap())

            r9, coeff, beta_sb = _rodrigues_and_coeff(
                nc, tc, constp, statep, scrp, pose_sb, beta_sb)

            # ---- coeffT8 [115, 2, 128] via two fp32 transposes + fp8 cast ----
            coeffT8 = statep.tile([KVH, 2, B_LOC], F8)
            betaT = statep.tile([10, B_LOC], F32)
            j_sb = statep.tile([B_LOC, 72], F32)
            with tc.tile_pool(name="psA", bufs=2, space="PSUM") as psA:
                ptA = psA.tile([KVH, 128], F32, tag="tp")
                nc.tensor.transpose(ptA[:, :], coeff[:, 0:KVH], ident[:, :])
                V.tensor_copy(coeffT8[:, 0, :], ptA[:, 0:B_LOC])
                V.tensor_scalar_mul(betaT[:, :], ptA[0:10, 0:B_LOC], 16.0)
                ptB = psA.tile([KVH, 128], F32, tag="tp")
                nc.tensor.transpose(ptB[:, :], coeff[:, KVH:KVP], ident[:, :])
                V.tensor_copy(coeffT8[:, 1, :], ptB[:, 0:B_LOC])

                # ---- J = [beta | 1] @ [JS2 ; Jtmpl] ----
                pj = psA.tile([B_LOC, 72], F32, tag="pj")
                onesT = statep.tile([1, B_LOC], F32)
                V.memset(onesT[0:1, :], 1.0)
                nc.tensor.matmul(pj[:, :], betaT[:, :], js2[:, :], start=True, stop=False)
                nc.tensor.matmul(pj[:, :], onesT[0:1, :], jtmpl[0:1, :], start=False, stop=True)
                V.tensor_copy(j_sb[:, :], pj[:, :])

            # ---- P1: vp matmuls (independent of FK; emitted before gat
            # transposes so the PE queue is not blocked behind FK).
            # _p1_chunk is invoked for chunks 0..TSPLIT-1 here and the rest
            # after the gat transposes, so the transposes (which wait on FK)
            # slot into the PE queue right when FK finishes. ----
            vp_sb = statep.tile([B_LOC, 3, NVP], F16)
            p1_ctx = ctx = tc.tile_pool(name="p1", bufs=4)
            p1p = ctx.__enter__()
            # six 1-bank psum tiles with per-plane S copies: each copy
            # releases its bank right after its matmul, so chunk k+2's mms
            # unblock ~1us earlier than with a single wide 3-bank tile
            # (P1 paced at 2.85us/chunk for 1.54us of matmul otherwise)
            psVP_ctx = tc.tile_pool(name="psVP", bufs=6, space="PSUM")
            psVP = psVP_ctx.__enter__()

            da8_pair = [None]

            def _p1_chunk(ci):
                sz = min(CH, NV - ci * CH)
                if ci % 2 == 0:
                    da8 = p1p.tile([KVH, 2, 2, 3, CH], F8, tag="da")
                    da8_pair[0] = da8
                    nc.sync.dma_start(da8[:, :, :, :, :],
                                      dirs8_d.ap()[ci // 2])
                da8 = da8_pair[0]
                for c in range(3):
                    pvc = psVP.tile([B_LOC, CH], F32, tag="vp")
                    nc.tensor.matmul(pvc[:, 0:sz], coeffT8[:, :, :],
                                     da8[:, ci % 2, :, c, 0:sz], start=True,
                                     stop=True, perf_mode=DR)
                    S.copy(vp_sb[:, c, ci * CH:ci * CH + sz], pvc[:, 0:sz])

            TSPLIT = 12
            for ci in range(TSPLIT):
                _p1_chunk(ci)

            # ---- J_rel ----
            jrel = statep.tile([B_LOC, 72], F32)
            jv = j_sb[:, :].rearrange("p (j c) -> p j c", c=3)
            jrv = jrel[:, :].rearrange("p (j c) -> p j c", c=3)
            V.tensor_copy(jrel[:, 0:3], j_sb[:, 0:3])
            V.tensor_sub(jrv[:, 1:4], jv[:, 1:4], jv[:, 0:1].broadcast_to([B_LOC, 3, 3]))
            V.tensor_sub(jrv[:, 4:12], jv[:, 4:12], jv[:, 1:9])
            V.tensor_sub(jrv[:, 12:15], jv[:, 12:15], jv[:, 9:10].broadcast_to([B_LOC, 3, 3]))
            V.tensor_sub(jrv[:, 15:18], jv[:, 15:18], jv[:, 12:15])
            V.tensor_sub(jrv[:, 18:24], jv[:, 18:24], jv[:, 16:22])

            # ---- local transforms Gl [128, 24*12] (3x4 row-major [R|t]) ----
            gl = statep.tile([B_LOC, NJ * 12], F32)
            gl4 = gl[:, :].rearrange("p (j m n) -> p j m n", m=3, n=4)
            r94 = r9[:, :].rearrange("p (j m n) -> p j m n", m=3, n=3)
            V.tensor_copy(gl4[:, :, :, 0:3], r94[:, :, :, :])
            V.tensor_copy(gl4[:, :, :, 3:4], jrv[:, :, :].unsqueeze(3))

            # ---- forward kinematics ----
            gw = statep.tile([B_LOC, NJ * 12], F32)
            gw4 = gw[:, :].rearrange("p (j m n) -> p j m n", m=3, n=4)
            V.tensor_copy(gw[:, 0:12], gl[:, 0:12])
            fktmp = scrp.tile([B_LOC, 3 * 12], F32, tag="fktmp")
            for (c0, ncld, p0, bc) in FK_GROUPS:
                child = gw4[:, c0:c0 + ncld]
                loc = gl4[:, c0:c0 + ncld]
                par = gw4[:, p0:p0 + (1 if bc else ncld)]
                tmpv = fktmp[:, 0:ncld * 12].rearrange("p (j m n) -> p j m n", m=3, n=4)
                shp = [B_LOC, ncld, 3, 4]
                for k in range(3):
                    in0 = loc[:, :, k:k + 1, :].broadcast_to(shp)
                    pk = par[:, 0:1, :, k:k + 1] if bc else par[:, :, :, k:k + 1]
                    in1 = pk.broadcast_to(shp)
                    if k == 0:
                        V.tensor_mul(child[:, :, :, :], in0, in1)
                    else:
                        V.tensor_mul(tmpv, in0, in1)
                        V.tensor_add(child[:, :, :, :], child[:, :, :, :], tmpv)
                ptr = par[:, 0:1, :, 3:4] if bc else par[:, :, :, 3:4]
                V.tensor_add(child[:, :, :, 3:4], child[:, :, :, 3:4],
                             ptr.broadcast_to([B_LOC, ncld, 3, 1]))

            # ---- rest-pose correction: t_j -= R_j^w @ J_j ----
            ct = scrp.tile([B_LOC, 72], F32, tag="ct")
            ct2 = scrp.tile([B_LOC, 72], F32, tag="ct2")
            ctv = ct[:, :].rearrange("p (j m) -> p j m", m=3).unsqueeze(3)
            ct2v = ct2[:, :].rearrange("p (j m) -> p j m", m=3).unsqueeze(3)
            for k in range(3):
                jk = jv[:, :, k:k + 1].unsqueeze(2).broadcast_to([B_LOC, NJ, 3, 1])
                if k == 0:
                    V.tensor_mul(ctv, gw4[:, :, :, k:k + 1], jk)
                else:
                    V.tensor_mul(ct2v, gw4[:, :, :, k:k + 1], jk)
                    V.tensor_add(ctv, ctv, ct2v)
            V.tensor_sub(gw4[:, :, :, 3:4], gw4[:, :, :, 3:4], ctv)

            if cfg["debug"]:
                nc.sync.dma_start(dbg["r9"].ap(), r9[:, :])
                nc.sync.dma_start(dbg["j"].ap(), j_sb[:, :])
                nc.sync.dma_start(dbg["gw"].ap(), gw[:, :])

            # ---- gat16 via 12 fp32 transposes: [24, 12, 128] fp16 ----
            gat16 = statep.tile([NJ, 12, B_LOC], F16)
            gwe = gw[:, :].rearrange("p (j e) -> p e j", e=12)
            with tc.tile_pool(name="psT", bufs=2, space="PSUM") as psT:
                for e in range(12):
                    pgt = psT.tile([NJ, B_LOC], F32, tag="gt")
                    nc.tensor.transpose(pgt[:, :], gwe[:, e, :], ident[:, :])
                    V.tensor_copy(gat16[:, e, :], pgt[:, :])
            # remaining P1 chunks fill the PE queue behind the transposes
            for ci in range(TSPLIT, NCH):
                _p1_chunk(ci)
            psVP_ctx.__exit__(None, None, None)
            p1_ctx.__exit__(None, None, None)

            if cfg["debug"]:
                nc.sync.dma_start(dbg["vp"].ap(), vp_sb[:, :, :])

            # ---- P2: skinning matmuls + combine ----
            with (
                tc.tile_pool(name="p2", bufs=3) as p2p,
                tc.tile_pool(name="psTG", bufs=2, space="PSUM") as psTG,
            ):
                for ci in range(NCH):
                    v0 = ci * CH
                    sz = min(CH, NV - v0)
                    wt16c = p2p.tile([NJ, CH], F16, tag="wt")
                    nc.sync.dma_start(wt16c[:, :], wt16_d.ap()[ci])

                    t_sbs = []
                    for n in range(3):
                        ptn = psTG.tile([B_LOC, 3, CH], F32, tag="tg")
                        for m in range(3):
                            e = m * 4 + n
                            nc.tensor.matmul(ptn[:, m, 0:sz], gat16[:, e, :],
                                             wt16c[:, 0:sz], start=True, stop=True)
                        t_sb = p2p.tile([B_LOC, 3, CH], F16, tag=f"tsb{n}")
                        S.copy(t_sb[:, :, 0:sz], ptn[:, :, 0:sz])
                        t_sbs.append(t_sb)

                    pt3 = psTG.tile([B_LOC, 3, CH], F32, tag="tg")
                    for m in range(3):
                        e = m * 4 + 3
                        nc.tensor.matmul(pt3[:, m, 0:sz], gat16[:, e, :],
                                         wt16c[:, 0:sz], start=True, stop=True)
                    # pt3 evicted by S too: frees its psum slot fast (PE would
                    # otherwise stall on the rotation) and keeps the V add in
                    # fp16 2x mode. GpSimd is NOT used: it shares the DVE SBUF
                    # port, halving any concurrent 2-port V op (measured
                    # 950ns -> 3200ns).
                    pt3_sb = p2p.tile([B_LOC, 3, CH], F16, tag="pt3sb")
                    S.copy(pt3_sb[:, :, 0:sz], pt3[:, :, 0:sz])

                    tmps = []
                    for n in range(3):
                        tmp = p2p.tile([B_LOC, 3, CH], F16, tag=f"tmp{n}")
                        vb = vp_sb[:, n, v0:v0 + sz].unsqueeze(1).broadcast_to(
                            [B_LOC, 3, sz])
                        V.tensor_mul(tmp[:, :, 0:sz], t_sbs[n][:, :, 0:sz], vb)
                        tmps.append(tmp)
                    acc = p2p.tile([B_LOC, 3, CH], F16, tag="acc")
                    V.tensor_add(acc[:, :, 0:sz], tmps[0][:, :, 0:sz],
                                 pt3_sb[:, :, 0:sz])
                    acc2 = p2p.tile([B_LOC, 3, CH], F16, tag="acc2")
                    V.tensor_add(acc2[:, :, 0:sz], acc[:, :, 0:sz],
                                 tmps[1][:, :, 0:sz])
                    out_sb = p2p.tile([B_LOC, 3, CH], F16, tag="outsb")
                    V.tensor_add(out_sb[:, :, 0:sz], acc2[:, :, 0:sz],
                                 tmps[2][:, :, 0:sz])
                    nc.sync.dma_start(out_d.ap()[ci][:, :, 0:sz],
                                      out_sb[:, :, 0:sz])

    nc.compile()
    _CACHE[key] = nc
    return nc


def _host_prep_fp8(inputs):
    f32 = np.float32
    shapedirs = np.asarray(inputs["shapedirs"], f32)   # [V,3,10]
    posedirs = np.asarray(inputs["posedirs"], f32)     # [V,3,207]
    v_template = np.asarray(inputs["v_template"], f32)  # [V,3]
    Jreg = np.asarray(inputs["J_regressor"], f32)       # [24,V]
    weights = np.asarray(inputs["weights"], f32)        # [V,24]

    dirs = np.zeros((KVP, 3, NVP), f32)
    sd = shapedirs.transpose(2, 1, 0)   # [10,3,V]
    pd = posedirs.transpose(2, 1, 0)    # [207,3,V]
    dirs[0:10, :, :NV] = sd * 16.0
    dirs[10:217, :, :NV] = pd * 16.0
    tmpl = v_template.T
    hi8 = tmpl.astype(F8NP).astype(f32)
    dirs[217, :, :NV] = hi8
    dirs[218, :, :NV] = (tmpl - hi8) * 16.0
    dirs8 = dirs.astype(F8NP)
    d = dirs8.reshape(KVP, 3, NCH, CH)
    dirs8_arr = np.empty((NCH, KVH, 2, 3, CH), F8NP)
    dirs8_arr[:, :, 0] = d[0:KVH].transpose(2, 0, 1, 3)
    dirs8_arr[:, :, 1] = d[KVH:KVP].transpose(2, 0, 1, 3)
    # [NCH,...] -> chunk-pair-major [NCH/2, KVH, 2(chunk), 2(panel), 3, CH]
    dirs8_arr = dirs8_arr.reshape(NCH // 2, 2, KVH, 2, 3, CH).transpose(
        0, 2, 1, 3, 4, 5)

    wt = np.zeros((NJ, NVP), np.float16)
    wt[:, :NV] = weights.T.astype(np.float16)
    wt16_arr = np.ascontiguousarray(wt.reshape(NJ, NCH, CH).transpose(1, 0, 2))

    js2 = np.einsum('jv,vcs->sjc', Jreg, shapedirs).reshape(10, 72)
    jtmpl = (Jreg @ v_template).reshape(1, 72)
    return {
        "dirs8": np.ascontiguousarray(dirs8_arr),
        "wt16": wt16_arr,
        "js2": np.ascontiguousarray(js2),
        "jtmpl": np.ascontiguousarray(jtmpl),
        "ident": np.eye(128, dtype=f32),
    }


def kernel(pose, beta, shapedirs, posedirs, v_template, J_regressor, weights):
    cfg = CFG
    if cfg["compute"] == "fp8":
        nc = build_program_fp8(cfg)
        rep = _host_prep_fp8(dict(shapedirs=shapedirs, posedirs=posedirs,
                                  v_template=v_template, J_regressor=J_regressor,
                                  weights=weights))
    else:
        nc = build_program(cfg)
        rep = _host_prep(dict(shapedirs=shapedirs, posedirs=posedirs,
                              v_template=v_template, J_regressor=J_regressor,
                              weights=weights), cfg)
    pose = np.asarray(pose, np.float32)
    beta = np.asarray(beta, np.float32)
    in_maps = []
    for i in range(N_CORES):
        m = dict(rep)
        m["pose"] = np.ascontiguousarray(pose[i * B_LOC:(i + 1) * B_LOC])
        m["beta"] = np.ascontiguousarray(beta[i * B_LOC:(i + 1) * B_LOC])
        in_maps.append(m)
    res = run_bass_kernel_spmd(nc, in_maps, core_ids=list(range(N_CORES)),
                               trace=cfg.get("trace", False))
    kernel.last_results = res
    outs = []
    for i in range(N_CORES):
        o = np.asarray(res.results[i]["out"], np.float32)
        if cfg["compute"] == "fp8":
            # [NCH, 128, 3, CH] -> [128, 3, NVP] -> [128, NV, 3]
            o = o.transpose(1, 2, 0, 3).reshape(B_LOC, 3, NVP)[:, :, :NV]
        outs.append(o.transpose(0, 2, 1))
    return np.ascontiguousarray(np.concatenate(outs, axis=0))


# ---------------------------------------------------------------------------
# Legacy fp16 path (kept for A/B testing via CFG["compute"]="fp16")
# ---------------------------------------------------------------------------
KC = 218
VC = 3 * NV


def build_program(cfg):
    key = (cfg["compute"], cfg["ch"], cfg["out16"], cfg["debug"])
    if key in _CACHE:
        return _CACHE[key]

    fp16 = cfg["compute"] == "fp16"
    CDT = F16 if fp16 else F32
    ODT = F16 if (fp16 and cfg["out16"]) else F32
    ch = cfg["ch"] if fp16 else 256

    nc = bacc.Bacc("TRN2", target_bir_lowering=False, debug=False)

    pose_d = nc.dram_tensor("pose", [B_LOC, 72], F32, kind="ExternalInput")
    beta_d = nc.dram_tensor("beta", [B_LOC, 10], F32, kind="ExternalInput")
    dirs_d = nc.dram_tensor("dirs", [KC, VC], CDT, kind="ExternalInput")
    wt_d = nc.dram_tensor("wt", [NJ, NV], CDT, kind="ExternalInput")
    js2_d = nc.dram_tensor("js2", [10, 72], F32, kind="ExternalInput")
    jtmpl_d = nc.dram_tensor("jtmpl", [1, 72], F32, kind="ExternalInput")
    ident_d = nc.dram_tensor("ident", [128, 128], F32, kind="ExternalInput")
    out_d = nc.dram_tensor("out", [B_LOC, 3, NV], ODT, kind="ExternalOutput")
    dbg = {}
    if cfg["debug"]:
        dbg["r9"] = nc.dram_tensor("dbg_r9", [B_LOC, 216], F32, kind="ExternalOutput")
        dbg["j"] = nc.dram_tensor("dbg_j", [B_LOC, 72], F32, kind="ExternalOutput")
        dbg["gw"] = nc.dram_tensor("dbg_gw", [B_LOC, 288], F32, kind="ExternalOutput")
        dbg["vp"] = nc.dram_tensor("dbg_vp", [B_LOC, 3, NV], F32, kind="ExternalOutput")

    with tile.TileContext(nc) as tc:
        with (
            tc.tile_pool(name="const", bufs=1) as constp,
            tc.tile_pool(name="state", bufs=1) as statep,
            tc.tile_pool(name="scr", bufs=1) as scrp,
        ):
            ident = constp.tile([128, 128], F32)
            nc.sync.dma_start(ident[:, :], ident_d.ap())
            wt_sb = constp.tile([NJ, NV], CDT)
            nc.sync.dma_start(wt_sb[:, :], wt_d.ap())
            js2 = statep.tile([10, 72], F32)
            nc.sync.dma_start(js2[:, :], js2_d.ap())
            jtmpl = statep.tile([1, 72], F32)
            nc.sync.dma_start(jtmpl[:, :], jtmpl_d.ap())
            pose_sb = statep.tile([B_LOC, 72], F32)
            nc.sync.dma_start(pose_sb[:, :], pose_d.ap())

            V = nc.vector
            S = nc.scalar
            sq = scrp.tile([B_LOC, 72], F32, tag="sq")
            V.tensor_mul(sq[:, :], pose_sb[:, :], pose_sb[:, :])
            sq3 = sq[:, :].rearrange("p (j c) -> p c j", c=3)
            th2 = scrp.tile([B_LOC, NJ], F32, tag="th2")
            V.tensor_add(th2[:, :], sq3[:, 0, :], sq3[:, 1, :])
            V.tensor_add(th2[:, :], th2[:, :], sq3[:, 2, :])
            cbias = constp.tile([128, 2], F32)
            V.memset(cbias[:, 0:1], 1e-8)
            V.memset(cbias[:, 1:2], float(np.pi / 2))
            theta = scrp.tile([B_LOC, NJ], F32, tag="theta")
            S.activation(theta[:, :], th2[:, :], mybir.ActivationFunctionType.Sqrt,
                         bias=cbias[0:B_LOC, 0:1])
            invt = scrp.tile([B_LOC, NJ], F32, tag="invt")
            V.reciprocal(invt[:, :], theta[:, :])
            sh = scrp.tile([B_LOC, NJ], F32, tag="sh")
            S.activation(sh[:, :], theta[:, :], mybir.ActivationFunctionType.Sin, scale=0.5)
            chh = scrp.tile([B_LOC, NJ], F32, tag="chh")
            S.activation(chh[:, :], theta[:, :], mybir.ActivationFunctionType.Sin,
                         scale=0.5, bias=cbias[0:B_LOC, 1:2])
            s_t = scrp.tile([B_LOC, NJ], F32, tag="s_t")
            V.scalar_tensor_tensor(s_t[:, :], sh[:, :], 2.0, chh[:, :], AluOpType.mult, AluOpType.mult)
            shsq = scrp.tile([B_LOC, NJ], F32, tag="shsq")
            V.tensor_mul(shsq[:, :], sh[:, :], sh[:, :])
            c_t = scrp.tile([B_LOC, NJ], F32, tag="c_t")
            V.tensor_scalar(c_t[:, :], shsq[:, :], -2.0, 1.0, AluOpType.mult, AluOpType.add)
            omc = scrp.tile([B_LOC, NJ], F32, tag="omc")
            V.tensor_scalar_mul(omc[:, :], shsq[:, :], 2.0)
            ax = scrp.tile([B_LOC, 72], F32, tag="ax")
            ax3 = ax[:, :].rearrange("p (j c) -> p c j", c=3)
            p3 = pose_sb[:, :].rearrange("p (j c) -> p c j", c=3)
            for ci in range(3):
                V.tensor_mul(ax3[:, ci, :], p3[:, ci, :], invt[:, :])
            prods = {}
            for name, (a, b_) in {
                "xx": (0, 0), "yy": (1, 1), "zz": (2, 2),
                "xy": (0, 1), "xz": (0, 2), "yz": (1, 2),
            }.items():
                t = scrp.tile([B_LOC, NJ], F32, tag="prod_" + name)
                V.tensor_mul(t[:, :], ax3[:, a, :], ax3[:, b_, :])
                V.tensor_mul(t[:, :], t[:, :], omc[:, :])
                prods[name] = t
            for name, a in {"sx": 0, "sy": 1, "sz": 2}.items():
                t = scrp.tile([B_LOC, NJ], F32, tag="prod_" + name)
                V.tensor_mul(t[:, :], s_t[:, :], ax3[:, a, :])
                prods[name] = t
            r9 = statep.tile([B_LOC, NJ * 9], F32)
            r9e = r9[:, :].rearrange("p (j e) -> p e j", e=9)
            ENTRIES = [
                ("add", "c", "xx"), ("sub", "xy", "sz"), ("add", "xz", "sy"),
                ("add", "xy", "sz"), ("add", "c", "yy"), ("sub", "yz", "sx"),
                ("sub", "xz", "sy"), ("add", "yz", "sx"), ("add", "c", "zz"),
            ]
            for e, (op, a, b_) in enumerate(ENTRIES):
                ta = c_t if a == "c" else prods[a]
                fn = V.tensor_add if op == "add" else V.tensor_sub
                fn(r9e[:, e, :], ta[:, :], prods[b_][:, :])

            coeff = statep.tile([B_LOC, KC], F32)
            nc.sync.dma_start(coeff[:, 0:10], beta_d.ap())
            V.tensor_copy(coeff[:, 10:217], r9[:, 9:216])
            lr9 = coeff[:, 10:217].rearrange("p (j e) -> p e j", e=9)
            for e in (0, 4, 8):
                V.tensor_scalar_add(lr9[:, e, :], lr9[:, e, :], -1.0)
            V.memset(coeff[:, 217:218], 1.0)

            with tc.tile_pool(name="psA", bufs=2, space="PSUM") as psA:
                pt1 = psA.tile([128, 128], F32, tag="tp")
                nc.tensor.transpose(pt1[:, :], coeff[:, 0:128], ident[:, :])
                coeffT_a = statep.tile([128, B_LOC], CDT)
                V.tensor_copy(coeffT_a[:, :], pt1[:, :])
                pt2 = psA.tile([128, 128], F32, tag="tp")
                nc.tensor.transpose(pt2[0:90, :], coeff[:, 128:218], ident[:, :])
                coeffT_b = statep.tile([90, B_LOC], CDT)
                V.tensor_copy(coeffT_b[:, :], pt2[0:90, :])

                pj = psA.tile([B_LOC, 72], F32, tag="pj")
                onesT = statep.tile([1, B_LOC], F32)
                V.memset(onesT[0:1, :], 1.0)
                if fp16:
                    betaT = statep.tile([10, B_LOC], F32)
                    V.tensor_copy(betaT[:, :], pt1[0:10, :])
                    betaT_ap = betaT[:, :]
                else:
                    betaT_ap = coeffT_a[0:10, :]
                nc.tensor.matmul(pj[:, :], betaT_ap, js2[:, :], start=True, stop=False)
                nc.tensor.matmul(pj[:, :], onesT[0:1, :], jtmpl[0:1, :], start=False, stop=True)
                j_sb = statep.tile([B_LOC, 72], F32)
                V.tensor_copy(j_sb[:, :], pj[:, :])

            jrel = statep.tile([B_LOC, 72], F32)
            jv = j_sb[:, :].rearrange("p (j c) -> p j c", c=3)
            jrv = jrel[:, :].rearrange("p (j c) -> p j c", c=3)
            V.tensor_copy(jrel[:, 0:3], j_sb[:, 0:3])
            V.tensor_sub(jrv[:, 1:4], jv[:, 1:4], jv[:, 0:1].broadcast_to([B_LOC, 3, 3]))
            V.tensor_sub(jrv[:, 4:12], jv[:, 4:12], jv[:, 1:9])
            V.tensor_sub(jrv[:, 12:15], jv[:, 12:15], jv[:, 9:10].broadcast_to([B_LOC, 3, 3]))
            V.tensor_sub(jrv[:, 15:18], jv[:, 15:18], jv[:, 12:15])
            V.tensor_sub(jrv[:, 18:24], jv[:, 18:24], jv[:, 16:22])

            gl = statep.tile([B_LOC, NJ * 12], F32)
            gl4 = gl[:, :].rearrange("p (j m n) -> p j m n", m=3, n=4)
            r94 = r9[:, :].rearrange("p (j m n) -> p j m n", m=3, n=3)
            V.tensor_copy(gl4[:, :, :, 0:3], r94[:, :, :, :])
            V.tensor_copy(gl4[:, :, :, 3:4], jrv[:, :, :].unsqueeze(3))

            gw = statep.tile([B_LOC, NJ * 12], F32)
            gw4 = gw[:, :].rearrange("p (j m n) -> p j m n", m=3, n=4)
            V.tensor_copy(gw[:, 0:12], gl[:, 0:12])
            fktmp = scrp.tile([B_LOC, 3 * 12], F32, tag="fktmp")
            for (c0, ncld, p0, bc) in FK_GROUPS:
                child = gw4[:, c0:c0 + ncld]
                loc = gl4[:, c0:c0 + ncld]
                par = gw4[:, p0:p0 + (1 if bc else ncld)]
                tmpv = fktmp[:, 0:ncld * 12].rearrange("p (j m n) -> p j m n", m=3, n=4)
                shp = [B_LOC, ncld, 3, 4]
                for k in range(3):
                    in0 = loc[:, :, k:k + 1, :].broadcast_to(shp)
                    pk = par[:, 0:1, :, k:k + 1] if bc else par[:, :, :, k:k + 1]
                    in1 = pk.broadcast_to(shp)
                    if k == 0:
                        V.tensor_mul(child[:, :, :, :], in0, in1)
                    else:
                        V.tensor_mul(tmpv, in0, in1)
                        V.tensor_add(child[:, :, :, :], child[:, :, :, :], tmpv)
                ptr = par[:, 0:1, :, 3:4] if bc else par[:, :, :, 3:4]
                V.tensor_add(child[:, :, :, 3:4], child[:, :, :, 3:4],
                             ptr.broadcast_to([B_LOC, ncld, 3, 1]))

            ct = scrp.tile([B_LOC, 72], F32, tag="ct")
            ct2 = scrp.tile([B_LOC, 72], F32, tag="ct2")
            ctv = ct[:, :].rearrange("p (j m) -> p j m", m=3).unsqueeze(3)
            ct2v = ct2[:, :].rearrange("p (j m) -> p j m", m=3).unsqueeze(3)
            for k in range(3):
                jk = jv[:, :, k:k + 1].unsqueeze(2).broadcast_to([B_LOC, NJ, 3, 1])
                if k == 0:
                    V.tensor_mul(ctv, gw4[:, :, :, k:k + 1], jk)
                else:
                    V.tensor_mul(ct2v, gw4[:, :, :, k:k + 1], jk)
                    V.tensor_add(ctv, ctv, ct2v)
            V.tensor_sub(gw4[:, :, :, 3:4], gw4[:, :, :, 3:4], ctv)

            if cfg["debug"]:
                nc.sync.dma_start(dbg["r9"].ap(), r9[:, :])
                nc.sync.dma_start(dbg["j"].ap(), j_sb[:, :])
                nc.sync.dma_start(dbg["gw"].ap(), gw[:, :])

            gat = statep.tile([NJ, 12 * B_LOC], CDT)
            gwe = gw[:, :].rearrange("p (j e) -> p e j", e=12)
            with tc.tile_pool(name="psT", bufs=3, space="PSUM") as psT:
                for e in range(12):
                    pgt = psT.tile([NJ, B_LOC], F32, tag="gt")
                    nc.tensor.transpose(pgt[:, :], gwe[:, e, :], ident[:, :])
                    V.tensor_copy(gat[:, e * B_LOC:(e + 1) * B_LOC], pgt[:, :])

            dirs_ap = dirs_d.ap().rearrange("k (c v) -> k c v", c=3)
            _main_loop_v2(nc, tc, cfg, ch, ODT, dirs_ap, coeffT_a, coeffT_b, gat,
                          wt_sb, out_d)

    nc.compile()
    _CACHE[key] = nc
    return nc


def _main_loop_v2(nc, tc, cfg, ch, ODT, dirs_ap, coeffT_a, coeffT_b, gat, wt_sb, out_d):
    V = nc.vector
    S = nc.scalar
    P = nc.gpsimd
    n_chunks = (NV + ch - 1) // ch
    with (
        tc.tile_pool(name="loop", bufs=3) as loopp,
        tc.tile_pool(name="psMM", bufs=2, space="PSUM") as psMM,
        tc.tile_pool(name="psTG", bufs=2, space="PSUM") as psTG,
    ):
        for ci in range(n_chunks):
            v0 = ci * ch
            sz = min(ch, NV - v0)
            da = loopp.tile([128, 3, ch], F16, tag="da")
            nc.sync.dma_start(da[:, :, 0:sz], dirs_ap[0:128, :, v0:v0 + sz])
            db = loopp.tile([90, 3, ch], F16, tag="db")
            nc.sync.dma_start(db[:, :, 0:sz], dirs_ap[128:KC, :, v0:v0 + sz])

            vp_sb = loopp.tile([B_LOC, 3, ch], F16, tag="vp")
            for c in range(3):
                pvc = psMM.tile([B_LOC, ch], F32, tag="mm")
                nc.tensor.matmul(pvc[:, 0:sz], coeffT_a[:, :], da[:, c, 0:sz],
                                 start=True, stop=False)
                nc.tensor.matmul(pvc[:, 0:sz], coeffT_b[:, :], db[:, c, 0:sz],
                                 start=False, stop=True)
                S.copy(vp_sb[:, c, 0:sz], pvc[:, 0:sz])

            t_sbs = []
            for n in range(3):
                ptn = psTG.tile([B_LOC, 3, ch], F32, tag="tg")
                for m in range(3):
                    e = m * 4 + n
                    nc.tensor.matmul(ptn[:, m, 0:sz],
                                     gat[:, e * B_LOC:(e + 1) * B_LOC],
                                     wt_sb[:, v0:v0 + sz], start=True, stop=True)
                t_sb = loopp.tile([B_LOC, 3, ch], F16, tag=f"tsb{n}")
                if sz == ch:
                    S.copy(t_sb[:, :, :], ptn[:, :, :])
                else:
                    for m in range(3):
                        S.copy(t_sb[:, m, 0:sz], ptn[:, m, 0:sz])
                t_sbs.append(t_sb)

            pt3 = psTG.tile([B_LOC, 3, ch], F32, tag="tg")
            for m in range(3):
                e = m * 4 + 3
                nc.tensor.matmul(pt3[:, m, 0:sz],
                                 gat[:, e * B_LOC:(e + 1) * B_LOC],
                                 wt_sb[:, v0:v0 + sz], start=True, stop=True)

            tmps = []
            for n in range(3):
                tmp = loopp.tile([B_LOC, 3, ch], F16, tag=f"tmp{n}")
                vb = vp_sb[:, n, 0:sz].unsqueeze(1).broadcast_to([B_LOC, 3, sz])
                eng = P if n == 2 else V
                eng.tensor_mul(tmp[:, :, 0:sz], t_sbs[n][:, :, 0:sz], vb)
                tmps.append(tmp)
            acc = loopp.tile([B_LOC, 3, ch], F16, tag="acc")
            V.tensor_add(acc[:, :, 0:sz], tmps[0][:, :, 0:sz], pt3[:, :, 0:sz])
            V.tensor_add(acc[:, :, 0:sz], acc[:, :, 0:sz], tmps[1][:, :, 0:sz])
            out_sb = loopp.tile([B_LOC, 3, ch], ODT, tag="outsb")
            V.tensor_add(out_sb[:, :, 0:sz], acc[:, :, 0:sz], tmps[2][:, :, 0:sz])
            nc.sync.dma_start(out_d.ap()[:, :, v0:v0 + sz], out_sb[:, :, 0:sz])


def _host_prep(inputs, cfg):
    fp16 = cfg["compute"] == "fp16"
    cdt = np.float16 if fp16 else np.float32
    shapedirs = np.asarray(inputs["shapedirs"], np.float32)
    posedirs = np.asarray(inputs["posedirs"], np.float32)
    v_template = np.asarray(inputs["v_template"], np.float32)
    Jreg = np.asarray(inputs["J_regressor"], np.float32)
    weights = np.asarray(inputs["weights"], np.float32)

    dirs = np.empty((KC, VC), np.float32)
    dirs[0:10] = shapedirs.transpose(2, 1, 0).reshape(10, VC)
    dirs[10:217] = posedirs.transpose(2, 1, 0).reshape(NP, VC)
    dirs[217] = v_template.T.reshape(VC)
    js2 = np.einsum('jv,vcs->sjc', Jreg, shapedirs).reshape(10, 72)
    jtmpl = (Jreg @ v_template).reshape(1, 72)
    return {
        "dirs": np.ascontiguousarray(dirs.astype(cdt)),
        "wt": np.ascontiguousarray(weights.T.astype(cdt)),
        "js2": np.ascontiguousarray(js2),
        "jtmpl": np.ascontiguousarray(jtmpl),
        "ident": np.eye(128, dtype=np.float32),
    }


# revision 51
# speedup vs baseline: 1.0280x; 1.0280x over previous
"""SMPL (shape blend + pose blend + LBS skinning) Bass kernel for 8 TRN2 NeuronCores.

Data-parallel over batch: B=1024 -> 128 per core. All SMPL buffers replicated.

Measured HW model this kernel is built around (NTFF traces on these cores):
  - PE runs at 1.2 GHz here (no HAM ramp observed), 1 psum column/cycle,
    out <= 512 fp32 cols per matmul (1 PSUM bank), ~300ns fixed cost per
    matmul + ~420ns LDWEIGHTS+gap, partially hidden by the queue.
  - fp8 DoubleRow streams 2 packed columns/cycle -> same out-column rate as
    fp16, but doubles K capacity per pass: used for the K=230 vp matmul
    (one mm instead of two per c-plane).
  - ACTIVATE (ScalarE) is 1x, ~(N+352)/1.2 ns; DVE fp16 tensor_tensor is 2x;
    a DVE op overlapping any GpSimd op drops to ~0.5x (shared SBUF port), so
    GpSimd is left idle on purpose.

Numerics: vp in fp8e4 DoubleRow with power-of-2 row scaling and a hi/lo
compensation split for v_template; skinning matmul T in fp16.
Measured rel err 7.7e-3 vs the 2e-2 gate.
  vp (K=220 packed into 110 rows x 2 panels):
      rows = [beta/16 x shapedirs*16 | lrot/16 x posedirs*16 |
              1 x tmpl_hi | 1/16 x tmpl_lo*16 | pad]
  (110-row DMA tiles are ~2x faster per byte than 115-row ones - the DMA
  rate is sharply sensitive to partition-row count; dirs loads are issued
  at chunk-PAIR granularity, 6KB per partition row.)

Phase structure (per core):
  prologue: ACT-table warmup at t=0; Rodrigues (V+S) -> scaled coeff ->
            fp32 transposes -> coeffT8 (fp8); J matmul; FK on V
  P1 loop (overlaps FK on V): 3 DR matmuls/chunk -> 1-bank vp psum tiles ->
            per-plane S copies into persistent vp_sb [128,3,7168] fp16. The
            12 gat transposes are emitted mid-P1 (TSPLIT=12) so the PE
            reaches them right as FK finishes; remaining P1 chunks keep the
            PE queue fed.
  P2 loop:  12 fp16 matmuls/chunk into 3-bank psum n-groups (bufs=2) +
            pt3 group; S evicts all 4 groups to fp16 (incl pt3 - frees the
            psum slot fast and keeps V in 2x mode); V: 3 broadcast muls +
            3-add chain (never in-place: dst==src DVE ops run 4x slower).
Last chunk runs at its true 234-col width. Output [14, 128, 3, 512] fp16
chunk-major; host reassembles to [1024, 6890, 3] fp32.
"""

import sys
import numpy as np
import ml_dtypes

for _p in ("/opt/trn_rl_repo",):
    if _p not in sys.path:
        sys.path.append(_p)

import concourse.bass as bass
import concourse.tile as tile
import concourse.mybir as mybir
from concourse import bacc
from concourse.bass_utils import run_bass_kernel_spmd
from concourse.alu_op_type import AluOpType

F32 = mybir.dt.float32
F16 = mybir.dt.float16
F8 = mybir.dt.float8e4
F8NP = ml_dtypes.float8_e4m3
DR = mybir.MatmulPerfMode.DoubleRow

N_CORES = 8
B = 1024
B_LOC = B // N_CORES  # 128
NV = 6890
NVP = 7168            # padded to 14*512
NCH = 14
CH = 512
NJ = 24
NP = 207

KVP = 220             # vp contraction (padded even): 10+207+2+1
KVH = KVP // 2        # 110
KT = 64               # T contraction rows (x2 panels = 128 logical);
                      # blocks at 32-aligned bases: hi @0, lo8 @32, hi16 @0/p1

# FK level groups: (child_start, n_children, parent_start, parent_broadcast)
FK_GROUPS = [
    (1, 3, 0, True),
    (4, 3, 1, False),
    (7, 3, 4, False),
    (10, 3, 7, False),
    (13, 2, 9, True),
    (15, 3, 12, False),
    (18, 2, 16, False),
    (20, 2, 18, False),
    (22, 2, 20, False),
]

CFG = {
    "compute": "fp8",    # "fp8" | "fp16" (legacy)
    "ch": 512,
    "out16": True,
    "vp_fp8": False,
    "debug": False,
    "trace": False,
}

_CACHE = {}


def _rodrigues_and_coeff(nc, tc, constp, statep, scrp, pose_sb, beta_sb):
    """Rodrigues rotation build (fp32) + scaled coeff [128, KVP] fp32.

    coeff rows: 0..9 beta/16, 10..216 lrot/16, 217 = 1.0, 218 = 1/16,
    219 = 0.
    Returns (r9, coeff, beta_sb)."""
    V = nc.vector
    S = nc.scalar

    # dummy activation at t=0: pulls the Sqrt ACT-table load off the
    # Rodrigues critical path (it overlaps the pose DMA instead). Only the
    # Sqrt set is warmed: warming Sin too would thrash (the table RAM holds
    # one set at a time, so Sqrt would reload on the path; measured 4 loads
    # instead of 2).
    warm = scrp.tile([B_LOC, 1], F32, tag="warm")
    V.memset(warm[:, :], 1.0)
    S.activation(warm[:, :], warm[:, :], mybir.ActivationFunctionType.Sqrt)

    sq = scrp.tile([B_LOC, 72], F32, tag="sq")
    V.tensor_mul(sq[:, :], pose_sb[:, :], pose_sb[:, :])
    sq3 = sq[:, :].rearrange("p (j c) -> p c j", c=3)
    th2 = scrp.tile([B_LOC, NJ], F32, tag="th2")
    V.tensor_add(th2[:, :], sq3[:, 0, :], sq3[:, 1, :])
    V.tensor_add(th2[:, :], th2[:, :], sq3[:, 2, :])
    cbias = constp.tile([128, 2], F32)
    V.memset(cbias[:, 0:1], 1e-8)
    V.memset(cbias[:, 1:2], float(np.pi / 2))
    theta = scrp.tile([B_LOC, NJ], F32, tag="theta")
    S.activation(theta[:, :], th2[:, :], mybir.ActivationFunctionType.Sqrt,
                 bias=cbias[0:B_LOC, 0:1])
    invt = scrp.tile([B_LOC, NJ], F32, tag="invt")
    V.reciprocal(invt[:, :], theta[:, :])
    sh = scrp.tile([B_LOC, NJ], F32, tag="sh")
    S.activation(sh[:, :], theta[:, :], mybir.ActivationFunctionType.Sin, scale=0.5)
    chh = scrp.tile([B_LOC, NJ], F32, tag="chh")
    S.activation(chh[:, :], theta[:, :], mybir.ActivationFunctionType.Sin,
                 scale=0.5, bias=cbias[0:B_LOC, 1:2])
    s_t = scrp.tile([B_LOC, NJ], F32, tag="s_t")
    V.scalar_tensor_tensor(s_t[:, :], sh[:, :], 2.0, chh[:, :], AluOpType.mult, AluOpType.mult)
    shsq = scrp.tile([B_LOC, NJ], F32, tag="shsq")
    V.tensor_mul(shsq[:, :], sh[:, :], sh[:, :])
    c_t = scrp.tile([B_LOC, NJ], F32, tag="c_t")
    V.tensor_scalar(c_t[:, :], shsq[:, :], -2.0, 1.0, AluOpType.mult, AluOpType.add)
    omc = scrp.tile([B_LOC, NJ], F32, tag="omc")
    V.tensor_scalar_mul(omc[:, :], shsq[:, :], 2.0)
    ax = scrp.tile([B_LOC, 72], F32, tag="ax")
    ax3 = ax[:, :].rearrange("p (j c) -> p c j", c=3)
    p3 = pose_sb[:, :].rearrange("p (j c) -> p c j", c=3)
    V.tensor_mul(ax3[:, :, :], p3[:, :, :],
                 invt[:, :].unsqueeze(1).broadcast_to([B_LOC, 3, NJ]))
    # batched outer products: [xx yy zz xy xz yz sx sy sz] in 5 ops instead
    # of 15 (each ~180ns of serial V time on the coeffT8 critical chain)
    pr9 = scrp.tile([B_LOC, 9, NJ], F32, tag="pr9")
    prm = scrp.tile([B_LOC, 6, NJ], F32, tag="prm")
    V.tensor_mul(pr9[:, 0:3, :], ax3[:, 0:3, :], ax3[:, 0:3, :])
    V.tensor_mul(pr9[:, 3:5, :],
                 ax3[:, 0:1, :].broadcast_to([B_LOC, 2, NJ]), ax3[:, 1:3, :])
    V.tensor_mul(pr9[:, 5:6, :], ax3[:, 1:2, :], ax3[:, 2:3, :])
    V.tensor_mul(prm[:, :, :], pr9[:, 0:6, :],
                 omc[:, :].unsqueeze(1).broadcast_to([B_LOC, 6, NJ]))
    V.tensor_mul(pr9[:, 6:9, :],
                 s_t[:, :].unsqueeze(1).broadcast_to([B_LOC, 3, NJ]),
                 ax3[:, 0:3, :])
    prods = {"xx": prm[:, 0, :], "yy": prm[:, 1, :], "zz": prm[:, 2, :],
             "xy": prm[:, 3, :], "xz": prm[:, 4, :], "yz": prm[:, 5, :],
             "sx": pr9[:, 6, :], "sy": pr9[:, 7, :], "sz": pr9[:, 8, :]}
    r9 = statep.tile([B_LOC, NJ * 9], F32)
    r9e = r9[:, :].rearrange("p (j e) -> p e j", e=9)
    ENTRIES = [
        ("add", "c", "xx"), ("sub", "xy", "sz"), ("add", "xz", "sy"),
        ("add", "xy", "sz"), ("add", "c", "yy"), ("sub", "yz", "sx"),
        ("sub", "xz", "sy"), ("add", "yz", "sx"), ("add", "c", "zz"),
    ]
    for e, (op, a, b_) in enumerate(ENTRIES):
        ta = c_t[:, :] if a == "c" else prods[a]
        fn = V.tensor_add if op == "add" else V.tensor_sub
        fn(r9e[:, e, :], ta, prods[b_])

    # ---- scaled coeff ----
    coeff = statep.tile([B_LOC, KVP], F32)
    V.tensor_scalar_mul(coeff[:, 0:10], beta_sb[:, :], 1.0 / 16.0)
    # lrot/16 with diag -1/16
    V.tensor_scalar_mul(coeff[:, 10:217], r9[:, 9:216], 1.0 / 16.0)
    lr9 = coeff[:, 10:217].rearrange("p (j e) -> p e j", e=9)
    for e in (0, 4, 8):
        V.tensor_scalar_add(lr9[:, e, :], lr9[:, e, :], -1.0 / 16.0)
    V.memset(coeff[:, 217:218], 1.0)
    V.memset(coeff[:, 218:219], 1.0 / 16.0)
    V.memset(coeff[:, 219:220], 0.0)
    return r9, coeff, beta_sb


def build_program_fp8(cfg):
    key = ("fp8", cfg["debug"])
    if key in _CACHE:
        return _CACHE[key]

    nc = bacc.Bacc("TRN2", target_bir_lowering=False, debug=False)

    pose_d = nc.dram_tensor("pose", [B_LOC, 72], F32, kind="ExternalInput")
    beta_d = nc.dram_tensor("beta", [B_LOC, 10], F32, kind="ExternalInput")
    # chunk-PAIR granularity: 6144B per partition row per DMA (115 x 3KB
    # descriptors measured only ~23GB/s/engine; P1 was DMA-bandwidth bound
    # at 2.85us/chunk)
    dirs8_d = nc.dram_tensor("dirs8", [NCH // 2, KVH, 2, 2, 3, CH], F8,
                             kind="ExternalInput")
    # wt replicated at partition bases 0/32/64: T matmuls rotate across
    # three 32-row PE quadrants so LDWEIGHTS can be pulled ahead into idle
    # rows while the previous matmul streams (re-validated: the earlier
    # "regression" was device throttling; fast-equivalent T-mm is ~600 vs
    # 625ns)
    wt16_d = nc.dram_tensor("wt16", [NCH, 128, CH], F16, kind="ExternalInput")
    js2_d = nc.dram_tensor("js2", [10, 72], F32, kind="ExternalInput")
    jtmpl_d = nc.dram_tensor("jtmpl", [1, 72], F32, kind="ExternalInput")
    ident_d = nc.dram_tensor("ident", [128, 128], F32, kind="ExternalInput")
    out_d = nc.dram_tensor("out", [NCH, B_LOC, 3, CH], F16, kind="ExternalOutput")
    dbg = {}
    if cfg["debug"]:
        dbg["r9"] = nc.dram_tensor("dbg_r9", [B_LOC, 216], F32, kind="ExternalOutput")
        dbg["j"] = nc.dram_tensor("dbg_j", [B_LOC, 72], F32, kind="ExternalOutput")
        dbg["gw"] = nc.dram_tensor("dbg_gw", [B_LOC, 288], F32, kind="ExternalOutput")
        dbg["vp"] = nc.dram_tensor("dbg_vp", [B_LOC, 3, NVP], F16, kind="ExternalOutput")

    with tile.TileContext(nc) as tc:
        with (
            tc.tile_pool(name="const", bufs=1) as constp,
            tc.tile_pool(name="state", bufs=1) as statep,
            tc.tile_pool(name="scr", bufs=1) as scrp,
        ):
            V = nc.vector
            S = nc.scalar
            # pose/beta first on the sync queue: they gate the whole
            # Rodrigues critical chain (ident/js2/jtmpl aren't needed until
            # the transposes ~10us later)
            pose_sb = statep.tile([B_LOC, 72], F32)
            nc.sync.dma_start(pose_sb[:, :], pose_d.ap())
            beta_sb = statep.tile([B_LOC, 10], F32)
            nc.sync.dma_start(beta_sb[:, :], beta_d.ap())
            ident = constp.tile([128, 128], F32)
            nc.sync.dma_start(ident[:, :], ident_d.ap())
            js2 = statep.tile([10, 72], F32)
            nc.sync.dma_start(js2[:, :], js2_d.ap())
            jtmpl = statep.tile([1, 72], F32)
            nc.sync.dma_start(jtmpl[:, :], jtmpl_d.ap())

            r9, coeff, beta_sb = _rodrigues_and_coeff(
                nc, tc, constp, statep, scrp, pose_sb, beta_sb)

            # ---- coeffT8 [115, 2, 128] via two fp32 transposes + fp8 cast ----
            coeffT8 = statep.tile([KVH, 2, B_LOC], F8)
            betaT = statep.tile([10, B_LOC], F32)
            j_sb = statep.tile([B_LOC, 72], F32)
            with tc.tile_pool(name="psA", bufs=2, space="PSUM") as psA:
                ptA = psA.tile([KVH, 128], F32, tag="tp")
                nc.tensor.transpose(ptA[:, :], coeff[:, 0:KVH], ident[:, :])
                V.tensor_copy(coeffT8[:, 0, :], ptA[:, 0:B_LOC])
                V.tensor_scalar_mul(betaT[:, :], ptA[0:10, 0:B_LOC], 16.0)
                ptB = psA.tile([KVH, 128], F32, tag="tp")
                nc.tensor.transpose(ptB[:, :], coeff[:, KVH:KVP], ident[:, :])
                V.tensor_copy(coeffT8[:, 1, :], ptB[:, 0:B_LOC])

                # ---- J = [beta | 1] @ [JS2 ; Jtmpl] ----
                pj = psA.tile([B_LOC, 72], F32, tag="pj")
                onesT = statep.tile([1, B_LOC], F32)
                V.memset(onesT[0:1, :], 1.0)
                nc.tensor.matmul(pj[:, :], betaT[:, :], js2[:, :], start=True, stop=False)
                nc.tensor.matmul(pj[:, :], onesT[0:1, :], jtmpl[0:1, :], start=False, stop=True)
                V.tensor_copy(j_sb[:, :], pj[:, :])

            # ---- P1: vp matmuls (independent of FK; emitted before gat
            # transposes so the PE queue is not blocked behind FK).
            # _p1_chunk is invoked for chunks 0..TSPLIT-1 here and the rest
            # after the gat transposes, so the transposes (which wait on FK)
            # slot into the PE queue right when FK finishes. ----
            vp_sb = statep.tile([B_LOC, 3, NVP], F16)
            p1_ctx = ctx = tc.tile_pool(name="p1", bufs=4)
            p1p = ctx.__enter__()
            # six 1-bank psum tiles with per-plane S copies: each copy
            # releases its bank right after its matmul, so chunk k+2's mms
            # unblock ~1us earlier than with a single wide 3-bank tile
            # (P1 paced at 2.85us/chunk for 1.54us of matmul otherwise)
            psVP_ctx = tc.tile_pool(name="psVP", bufs=6, space="PSUM")
            psVP = psVP_ctx.__enter__()

            da8_pair = [None]

            def _p1_chunk(ci):
                sz = min(CH, NV - ci * CH)
                if ci % 2 == 0:
                    da8 = p1p.tile([KVH, 2, 2, 3, CH], F8, tag="da")
                    da8_pair[0] = da8
                    nc.sync.dma_start(da8[:, :, :, :, :],
                                      dirs8_d.ap()[ci // 2])
                da8 = da8_pair[0]
                for c in range(3):
                    pvc = psVP.tile([B_LOC, CH], F32, tag="vp")
                    nc.tensor.matmul(pvc[:, 0:sz], coeffT8[:, :, :],
                                     da8[:, ci % 2, :, c, 0:sz], start=True,
                                     stop=True, perf_mode=DR)
                    S.copy(vp_sb[:, c, ci * CH:ci * CH + sz], pvc[:, 0:sz])

            TSPLIT = 12
            for ci in range(TSPLIT):
                _p1_chunk(ci)

            # ---- J_rel ----
            jrel = statep.tile([B_LOC, 72], F32)
            jv = j_sb[:, :].rearrange("p (j c) -> p j c", c=3)
            jrv = jrel[:, :].rearrange("p (j c) -> p j c", c=3)
            V.tensor_copy(jrel[:, 0:3], j_sb[:, 0:3])
            V.tensor_sub(jrv[:, 1:4], jv[:, 1:4], jv[:, 0:1].broadcast_to([B_LOC, 3, 3]))
            V.tensor_sub(jrv[:, 4:12], jv[:, 4:12], jv[:, 1:9])
            V.tensor_sub(jrv[:, 12:15], jv[:, 12:15], jv[:, 9:10].broadcast_to([B_LOC, 3, 3]))
            V.tensor_sub(jrv[:, 15:18], jv[:, 15:18], jv[:, 12:15])
            V.tensor_sub(jrv[:, 18:24], jv[:, 18:24], jv[:, 16:22])

            # ---- local transforms Gl [128, 24*12] (3x4 row-major [R|t]) ----
            gl = statep.tile([B_LOC, NJ * 12], F32)
            gl4 = gl[:, :].rearrange("p (j m n) -> p j m n", m=3, n=4)
            r94 = r9[:, :].rearrange("p (j m n) -> p j m n", m=3, n=3)
            V.tensor_copy(gl4[:, :, :, 0:3], r94[:, :, :, :])
            V.tensor_copy(gl4[:, :, :, 3:4], jrv[:, :, :].unsqueeze(3))

            # ---- forward kinematics ----
            gw = statep.tile([B_LOC, NJ * 12], F32)
            gw4 = gw[:, :].rearrange("p (j m n) -> p j m n", m=3, n=4)
            V.tensor_copy(gw[:, 0:12], gl[:, 0:12])
            fktmp = scrp.tile([B_LOC, 3 * 12], F32, tag="fktmp")
            for (c0, ncld, p0, bc) in FK_GROUPS:
                child = gw4[:, c0:c0 + ncld]
                loc = gl4[:, c0:c0 + ncld]
                par = gw4[:, p0:p0 + (1 if bc else ncld)]
                tmpv = fktmp[:, 0:ncld * 12].rearrange("p (j m n) -> p j m n", m=3, n=4)
                shp = [B_LOC, ncld, 3, 4]
                for k in range(3):
                    in0 = loc[:, :, k:k + 1, :].broadcast_to(shp)
                    pk = par[:, 0:1, :, k:k + 1] if bc else par[:, :, :, k:k + 1]
                    in1 = pk.broadcast_to(shp)
                    if k == 0:
                        V.tensor_mul(child[:, :, :, :], in0, in1)
                    else:
                        V.tensor_mul(tmpv, in0, in1)
                        V.tensor_add(child[:, :, :, :], child[:, :, :, :], tmpv)
                ptr = par[:, 0:1, :, 3:4] if bc else par[:, :, :, 3:4]
                V.tensor_add(child[:, :, :, 3:4], child[:, :, :, 3:4],
                             ptr.broadcast_to([B_LOC, ncld, 3, 1]))

            # ---- rest-pose correction: t_j -= R_j^w @ J_j ----
            ct = scrp.tile([B_LOC, 72], F32, tag="ct")
            ct2 = scrp.tile([B_LOC, 72], F32, tag="ct2")
            ctv = ct[:, :].rearrange("p (j m) -> p j m", m=3).unsqueeze(3)
            ct2v = ct2[:, :].rearrange("p (j m) -> p j m", m=3).unsqueeze(3)
            for k in range(3):
                jk = jv[:, :, k:k + 1].unsqueeze(2).broadcast_to([B_LOC, NJ, 3, 1])
                if k == 0:
                    V.tensor_mul(ctv, gw4[:, :, :, k:k + 1], jk)
                else:
                    V.tensor_mul(ct2v, gw4[:, :, :, k:k + 1], jk)
                    V.tensor_add(ctv, ctv, ct2v)
            V.tensor_sub(gw4[:, :, :, 3:4], gw4[:, :, :, 3:4], ctv)

            if cfg["debug"]:
                nc.sync.dma_start(dbg["r9"].ap(), r9[:, :])
                nc.sync.dma_start(dbg["j"].ap(), j_sb[:, :])
                nc.sync.dma_start(dbg["gw"].ap(), gw[:, :])

            # ---- gat16 via 12 fp32 transposes: [24, 12, 128] fp16 ----
            gat16 = statep.tile([NJ, 12, B_LOC], F16)
            gwe = gw[:, :].rearrange("p (j e) -> p e j", e=12)
            with tc.tile_pool(name="psT", bufs=2, space="PSUM") as psT:
                for e in range(12):
                    pgt = psT.tile([NJ, B_LOC], F32, tag="gt")
                    nc.tensor.transpose(pgt[:, :], gwe[:, e, :], ident[:, :])
                    V.tensor_copy(gat16[:, e, :], pgt[:, :])
            # replicate gat at the three quadrant bases (base 96 is rejected
            # by bass) for the quadrant-rotated T matmuls
            gat16x = statep.tile([96, 12, B_LOC], F16)
            for q in range(3):
                nc.sync.dma_start(gat16x[32 * q:32 * q + NJ, :, :],
                                  gat16[:, :, :])
            # remaining P1 chunks fill the PE queue behind the transposes
            for ci in range(TSPLIT, NCH):
                _p1_chunk(ci)
            psVP_ctx.__exit__(None, None, None)
            p1_ctx.__exit__(None, None, None)

            if cfg["debug"]:
                nc.sync.dma_start(dbg["vp"].ap(), vp_sb[:, :, :])

            # ---- P2: skinning matmuls + combine ----
            with (
                tc.tile_pool(name="p2", bufs=3) as p2p,
                tc.tile_pool(name="psTG", bufs=2, space="PSUM") as psTG,
            ):
                qi = 0
                for ci in range(NCH):
                    v0 = ci * CH
                    sz = min(CH, NV - v0)
                    wt16c = p2p.tile([128, CH], F16, tag="wt")
                    nc.sync.dma_start(wt16c[:, :], wt16_d.ap()[ci])

                    t_sbs = []
                    for n in range(3):
                        ptn = psTG.tile([B_LOC, 3, CH], F32, tag="tg")
                        for m in range(3):
                            e = m * 4 + n
                            q = qi % 3
                            qi += 1
                            nc.tensor.matmul(
                                ptn[:, m, 0:sz],
                                gat16x[32 * q:32 * q + NJ, e, :],
                                wt16c[32 * q:32 * q + NJ, 0:sz],
                                start=True, stop=True)
                        t_sb = p2p.tile([B_LOC, 3, CH], F16, tag=f"tsb{n}")
                        S.copy(t_sb[:, :, 0:sz], ptn[:, :, 0:sz])
                        t_sbs.append(t_sb)

                    pt3 = psTG.tile([B_LOC, 3, CH], F32, tag="tg")
                    for m in range(3):
                        e = m * 4 + 3
                        q = qi % 3
                        qi += 1
                        nc.tensor.matmul(pt3[:, m, 0:sz],
                                         gat16x[32 * q:32 * q + NJ, e, :],
                                         wt16c[32 * q:32 * q + NJ, 0:sz],
                                         start=True, stop=True)
                    # pt3 evicted by S too: frees its psum slot fast (PE would
                    # otherwise stall on the rotation) and keeps the V add in
                    # fp16 2x mode. GpSimd is NOT used: it shares the DVE SBUF
                    # port, halving any concurrent 2-port V op (measured
                    # 950ns -> 3200ns).
                    pt3_sb = p2p.tile([B_LOC, 3, CH], F16, tag="pt3sb")
                    S.copy(pt3_sb[:, :, 0:sz], pt3[:, :, 0:sz])

                    tmps = []
                    for n in range(3):
                        tmp = p2p.tile([B_LOC, 3, CH], F16, tag=f"tmp{n}")
                        vb = vp_sb[:, n, v0:v0 + sz].unsqueeze(1).broadcast_to(
                            [B_LOC, 3, sz])
                        V.tensor_mul(tmp[:, :, 0:sz], t_sbs[n][:, :, 0:sz], vb)
                        tmps.append(tmp)
                    acc = p2p.tile([B_LOC, 3, CH], F16, tag="acc")
                    V.tensor_add(acc[:, :, 0:sz], tmps[0][:, :, 0:sz],
                                 pt3_sb[:, :, 0:sz])
                    acc2 = p2p.tile([B_LOC, 3, CH], F16, tag="acc2")
                    V.tensor_add(acc2[:, :, 0:sz], acc[:, :, 0:sz],
                                 tmps[1][:, :, 0:sz])
                    out_sb = p2p.tile([B_LOC, 3, CH], F16, tag="outsb")
                    V.tensor_add(out_sb[:, :, 0:sz], acc2[:, :, 0:sz],
                                 tmps[2][:, :, 0:sz])
                    nc.sync.dma_start(out_d.ap()[ci][:, :, 0:sz],
                                      out_sb[:, :, 0:sz])

    nc.compile()
    _CACHE[key] = nc
    return nc


def _host_prep_fp8(inputs):
    f32 = np.float32
    shapedirs = np.asarray(inputs["shapedirs"], f32)   # [V,3,10]
    posedirs = np.asarray(inputs["posedirs"], f32)     # [V,3,207]
    v_template = np.asarray(inputs["v_template"], f32)  # [V,3]
    Jreg = np.asarray(inputs["J_regressor"], f32)       # [24,V]
    weights = np.asarray(inputs["weights"], f32)        # [V,24]

    dirs = np.zeros((KVP, 3, NVP), f32)
    sd = shapedirs.transpose(2, 1, 0)   # [10,3,V]
    pd = posedirs.transpose(2, 1, 0)    # [207,3,V]
    dirs[0:10, :, :NV] = sd * 16.0
    dirs[10:217, :, :NV] = pd * 16.0
    tmpl = v_template.T
    hi8 = tmpl.astype(F8NP).astype(f32)
    dirs[217, :, :NV] = hi8
    dirs[218, :, :NV] = (tmpl - hi8) * 16.0
    dirs8 = dirs.astype(F8NP)
    d = dirs8.reshape(KVP, 3, NCH, CH)
    dirs8_arr = np.empty((NCH, KVH, 2, 3, CH), F8NP)
    dirs8_arr[:, :, 0] = d[0:KVH].transpose(2, 0, 1, 3)
    dirs8_arr[:, :, 1] = d[KVH:KVP].transpose(2, 0, 1, 3)
    # [NCH,...] -> chunk-pair-major [NCH/2, KVH, 2(chunk), 2(panel), 3, CH]
    dirs8_arr = dirs8_arr.reshape(NCH // 2, 2, KVH, 2, 3, CH).transpose(
        0, 2, 1, 3, 4, 5)

    wt = np.zeros((NJ, NVP), np.float16)
    wt[:, :NV] = weights.T.astype(np.float16)
    wtc = wt.reshape(NJ, NCH, CH).transpose(1, 0, 2)    # [NCH, NJ, CH]
    wt16_arr = np.zeros((NCH, 128, CH), np.float16)
    for q in range(3):
        wt16_arr[:, 32 * q:32 * q + NJ] = wtc
    wt16_arr = np.ascontiguousarray(wt16_arr)

    js2 = np.einsum('jv,vcs->sjc', Jreg, shapedirs).reshape(10, 72)
    jtmpl = (Jreg @ v_template).reshape(1, 72)
    return {
        "dirs8": np.ascontiguousarray(dirs8_arr),
        "wt16": wt16_arr,
        "js2": np.ascontiguousarray(js2),
        "jtmpl": np.ascontiguousarray(jtmpl),
        "ident": np.eye(128, dtype=f32),
    }


def kernel(pose, beta, shapedirs, posedirs, v_template, J_regressor, weights):
    cfg = CFG
    if cfg["compute"] == "fp8":
        nc = build_program_fp8(cfg)
        rep = _host_prep_fp8(dict(shapedirs=shapedirs, posedirs=posedirs,
                                  v_template=v_template, J_regressor=J_regressor,
                                  weights=weights))
    else:
        nc = build_program(cfg)
        rep = _host_prep(dict(shapedirs=shapedirs, posedirs=posedirs,
                              v_template=v_template, J_regressor=J_regressor,
                              weights=weights), cfg)
    pose = np.asarray(pose, np.float32)
    beta = np.asarray(beta, np.float32)
    in_maps = []
    for i in range(N_CORES):
        m = dict(rep)
        m["pose"] = np.ascontiguousarray(pose[i * B_LOC:(i + 1) * B_LOC])
        m["beta"] = np.ascontiguousarray(beta[i * B_LOC:(i + 1) * B_LOC])
        in_maps.append(m)
    res = run_bass_kernel_spmd(nc, in_maps, core_ids=list(range(N_CORES)),
                               trace=cfg.get("trace", False))
    kernel.last_results = res
    outs = []
    for i in range(N_CORES):
        o = np.asarray(res.results[i]["out"], np.float32)
        if cfg["compute"] == "fp8":
            # [NCH, 128, 3, CH] -> [128, 3, NVP] -> [128, NV, 3]
            o = o.transpose(1, 2, 0, 3).reshape(B_LOC, 3, NVP)[:, :, :NV]
        outs.append(o.transpose(0, 2, 1))
    return np.ascontiguousarray(np.concatenate(outs, axis=0))


# ---------------------------------------------------------------------------
# Legacy fp16 path (kept for A/B testing via CFG["compute"]="fp16")
# ---------------------------------------------------------------------------
KC = 218
VC = 3 * NV


def build_program(cfg):
    key = (cfg["compute"], cfg["ch"], cfg["out16"], cfg["debug"])
    if key in _CACHE:
        return _CACHE[key]

    fp16 = cfg["compute"] == "fp16"
    CDT = F16 if fp16 else F32
    ODT = F16 if (fp16 and cfg["out16"]) else F32
    ch = cfg["ch"] if fp16 else 256

    nc = bacc.Bacc("TRN2", target_bir_lowering=False, debug=False)

    pose_d = nc.dram_tensor("pose", [B_LOC, 72], F32, kind="ExternalInput")
    beta_d = nc.dram_tensor("beta", [B_LOC, 10], F32, kind="ExternalInput")
    dirs_d = nc.dram_tensor("dirs", [KC, VC], CDT, kind="ExternalInput")
    wt_d = nc.dram_tensor("wt", [NJ, NV], CDT, kind="ExternalInput")
    js2_d = nc.dram_tensor("js2", [10, 72], F32, kind="ExternalInput")
    jtmpl_d = nc.dram_tensor("jtmpl", [1, 72], F32, kind="ExternalInput")
    ident_d = nc.dram_tensor("ident", [128, 128], F32, kind="ExternalInput")
    out_d = nc.dram_tensor("out", [B_LOC, 3, NV], ODT, kind="ExternalOutput")
    dbg = {}
    if cfg["debug"]:
        dbg["r9"] = nc.dram_tensor("dbg_r9", [B_LOC, 216], F32, kind="ExternalOutput")
        dbg["j"] = nc.dram_tensor("dbg_j", [B_LOC, 72], F32, kind="ExternalOutput")
        dbg["gw"] = nc.dram_tensor("dbg_gw", [B_LOC, 288], F32, kind="ExternalOutput")
        dbg["vp"] = nc.dram_tensor("dbg_vp", [B_LOC, 3, NV], F32, kind="ExternalOutput")

    with tile.TileContext(nc) as tc:
        with (
            tc.tile_pool(name="const", bufs=1) as constp,
            tc.tile_pool(name="state", bufs=1) as statep,
            tc.tile_pool(name="scr", bufs=1) as scrp,
        ):
            ident = constp.tile([128, 128], F32)
            nc.sync.dma_start(ident[:, :], ident_d.ap())
            wt_sb = constp.tile([NJ, NV], CDT)
            nc.sync.dma_start(wt_sb[:, :], wt_d.ap())
            js2 = statep.tile([10, 72], F32)
            nc.sync.dma_start(js2[:, :], js2_d.ap())
            jtmpl = statep.tile([1, 72], F32)
            nc.sync.dma_start(jtmpl[:, :], jtmpl_d.ap())
            pose_sb = statep.tile([B_LOC, 72], F32)
            nc.sync.dma_start(pose_sb[:, :], pose_d.ap())

            V = nc.vector
            S = nc.scalar
            sq = scrp.tile([B_LOC, 72], F32, tag="sq")
            V.tensor_mul(sq[:, :], pose_sb[:, :], pose_sb[:, :])
            sq3 = sq[:, :].rearrange("p (j c) -> p c j", c=3)
            th2 = scrp.tile([B_LOC, NJ], F32, tag="th2")
            V.tensor_add(th2[:, :], sq3[:, 0, :], sq3[:, 1, :])
            V.tensor_add(th2[:, :], th2[:, :], sq3[:, 2, :])
            cbias = constp.tile([128, 2], F32)
            V.memset(cbias[:, 0:1], 1e-8)
            V.memset(cbias[:, 1:2], float(np.pi / 2))
            theta = scrp.tile([B_LOC, NJ], F32, tag="theta")
            S.activation(theta[:, :], th2[:, :], mybir.ActivationFunctionType.Sqrt,
                         bias=cbias[0:B_LOC, 0:1])
            invt = scrp.tile([B_LOC, NJ], F32, tag="invt")
            V.reciprocal(invt[:, :], theta[:, :])
            sh = scrp.tile([B_LOC, NJ], F32, tag="sh")
            S.activation(sh[:, :], theta[:, :], mybir.ActivationFunctionType.Sin, scale=0.5)
            chh = scrp.tile([B_LOC, NJ], F32, tag="chh")
            S.activation(chh[:, :], theta[:, :], mybir.ActivationFunctionType.Sin,
                         scale=0.5, bias=cbias[0:B_LOC, 1:2])
            s_t = scrp.tile([B_LOC, NJ], F32, tag="s_t")
            V.scalar_tensor_tensor(s_t[:, :], sh[:, :], 2.0, chh[:, :], AluOpType.mult, AluOpType.mult)
            shsq = scrp.tile([B_LOC, NJ], F32, tag="shsq")
            V.tensor_mul(shsq[:, :], sh[:, :], sh[:, :])
            c_t = scrp.tile([B_LOC, NJ], F32, tag="c_t")
            V.tensor_scalar(c_t[:, :], shsq[:, :], -2.0, 1.0, AluOpType.mult, AluOpType.add)
            omc = scrp.tile([B_LOC, NJ], F32, tag="omc")
            V.tensor_scalar_mul(omc[:, :], shsq[:, :], 2.0)
            ax = scrp.tile([B_LOC, 72], F32, tag="ax")
            ax3 = ax[:, :].rearrange("p (j c) -> p c j", c=3)
            p3 = pose_sb[:, :].rearrange("p (j c) -> p c j", c=3)
            for ci in range(3):
                V.tensor_mul(ax3[:, ci, :], p3[:, ci, :], invt[:, :])
            prods = {}
            for name, (a, b_) in {
                "xx": (0, 0), "yy": (1, 1), "zz": (2, 2),
                "xy": (0, 1), "xz": (0, 2), "yz": (1, 2),
            }.items():
                t = scrp.tile([B_LOC, NJ], F32, tag="prod_" + name)
                V.tensor_mul(t[:, :], ax3[:, a, :], ax3[:, b_, :])
                V.tensor_mul(t[:, :], t[:, :], omc[:, :])
                prods[name] = t
            for name, a in {"sx": 0, "sy": 1, "sz": 2}.items():
                t = scrp.tile([B_LOC, NJ], F32, tag="prod_" + name)
                V.tensor_mul(t[:, :], s_t[:, :], ax3[:, a, :])
                prods[name] = t
            r9 = statep.tile([B_LOC, NJ * 9], F32)
            r9e = r9[:, :].rearrange("p (j e) -> p e j", e=9)
            ENTRIES = [
                ("add", "c", "xx"), ("sub", "xy", "sz"), ("add", "xz", "sy"),
                ("add", "xy", "sz"), ("add", "c", "yy"), ("sub", "yz", "sx"),
                ("sub", "xz", "sy"), ("add", "yz", "sx"), ("add", "c", "zz"),
            ]
            for e, (op, a, b_) in enumerate(ENTRIES):
                ta = c_t if a == "c" else prods[a]
                fn = V.tensor_add if op == "add" else V.tensor_sub
                fn(r9e[:, e, :], ta[:, :], prods[b_][:, :])

            coeff = statep.tile([B_LOC, KC], F32)
            nc.sync.dma_start(coeff[:, 0:10], beta_d.ap())
            V.tensor_copy(coeff[:, 10:217], r9[:, 9:216])
            lr9 = coeff[:, 10:217].rearrange("p (j e) -> p e j", e=9)
            for e in (0, 4, 8):
                V.tensor_scalar_add(lr9[:, e, :], lr9[:, e, :], -1.0)
            V.memset(coeff[:, 217:218], 1.0)

            with tc.tile_pool(name="psA", bufs=2, space="PSUM") as psA:
                pt1 = psA.tile([128, 128], F32, tag="tp")
                nc.tensor.transpose(pt1[:, :], coeff[:, 0:128], ident[:, :])
                coeffT_a = statep.tile([128, B_LOC], CDT)
                V.tensor_copy(coeffT_a[:, :], pt1[:, :])
                pt2 = psA.tile([128, 128], F32, tag="tp")
                nc.tensor.transpose(pt2[0:90, :], coeff[:, 128:218], ident[:, :])
                coeffT_b = statep.tile([90, B_LOC], CDT)
                V.tensor_copy(coeffT_b[:, :], pt2[0:90, :])

                pj = psA.tile([B_LOC, 72], F32, tag="pj")
                onesT = statep.tile([1, B_LOC], F32)
                V.memset(onesT[0:1, :], 1.0)
                if fp16:
                    betaT = statep.tile([10, B_LOC], F32)
                    V.tensor_copy(betaT[:, :], pt1[0:10, :])
                    betaT_ap = betaT[:, :]
                else:
                    betaT_ap = coeffT_a[0:10, :]
                nc.tensor.matmul(pj[:, :], betaT_ap, js2[:, :], start=True, stop=False)
                nc.tensor.matmul(pj[:, :], onesT[0:1, :], jtmpl[0:1, :], start=False, stop=True)
                j_sb = statep.tile([B_LOC, 72], F32)
                V.tensor_copy(j_sb[:, :], pj[:, :])

            jrel = statep.tile([B_LOC, 72], F32)
            jv = j_sb[:, :].rearrange("p (j c) -> p j c", c=3)
            jrv = jrel[:, :].rearrange("p (j c) -> p j c", c=3)
            V.tensor_copy(jrel[:, 0:3], j_sb[:, 0:3])
            V.tensor_sub(jrv[:, 1:4], jv[:, 1:4], jv[:, 0:1].broadcast_to([B_LOC, 3, 3]))
            V.tensor_sub(jrv[:, 4:12], jv[:, 4:12], jv[:, 1:9])
            V.tensor_sub(jrv[:, 12:15], jv[:, 12:15], jv[:, 9:10].broadcast_to([B_LOC, 3, 3]))
            V.tensor_sub(jrv[:, 15:18], jv[:, 15:18], jv[:, 12:15])
            V.tensor_sub(jrv[:, 18:24], jv[:, 18:24], jv[:, 16:22])

            gl = statep.tile([B_LOC, NJ * 12], F32)
            gl4 = gl[:, :].rearrange("p (j m n) -> p j m n", m=3, n=4)
            r94 = r9[:, :].rearrange("p (j m n) -> p j m n", m=3, n=3)
            V.tensor_copy(gl4[:, :, :, 0:3], r94[:, :, :, :])
            V.tensor_copy(gl4[:, :, :, 3:4], jrv[:, :, :].unsqueeze(3))

            gw = statep.tile([B_LOC, NJ * 12], F32)
            gw4 = gw[:, :].rearrange("p (j m n) -> p j m n", m=3, n=4)
            V.tensor_copy(gw[:, 0:12], gl[:, 0:12])
            fktmp = scrp.tile([B_LOC, 3 * 12], F32, tag="fktmp")
            for (c0, ncld, p0, bc) in FK_GROUPS:
                child = gw4[:, c0:c0 + ncld]
                loc = gl4[:, c0:c0 + ncld]
                par = gw4[:, p0:p0 + (1 if bc else ncld)]
                tmpv = fktmp[:, 0:ncld * 12].rearrange("p (j m n) -> p j m n", m=3, n=4)
                shp = [B_LOC, ncld, 3, 4]
                for k in range(3):
                    in0 = loc[:, :, k:k + 1, :].broadcast_to(shp)
                    pk = par[:, 0:1, :, k:k + 1] if bc else par[:, :, :, k:k + 1]
                    in1 = pk.broadcast_to(shp)
                    if k == 0:
                        V.tensor_mul(child[:, :, :, :], in0, in1)
                    else:
                        V.tensor_mul(tmpv, in0, in1)
                        V.tensor_add(child[:, :, :, :], child[:, :, :, :], tmpv)
                ptr = par[:, 0:1, :, 3:4] if bc else par[:, :, :, 3:4]
                V.tensor_add(child[:, :, :, 3:4], child[:, :, :, 3:4],
                             ptr.broadcast_to([B_LOC, ncld, 3, 1]))

            ct = scrp.tile([B_LOC, 72], F32, tag="ct")
            ct2 = scrp.tile([B_LOC, 72], F32, tag="ct2")
            ctv = ct[:, :].rearrange("p (j m) -> p j m", m=3).unsqueeze(3)
            ct2v = ct2[:, :].rearrange("p (j m) -> p j m", m=3).unsqueeze(3)
            for k in range(3):
                jk = jv[:, :, k:k + 1].unsqueeze(2).broadcast_to([B_LOC, NJ, 3, 1])
                if k == 0:
                    V.tensor_mul(ctv, gw4[:, :, :, k:k + 1], jk)
                else:
                    V.tensor_mul(ct2v, gw4[:, :, :, k:k + 1], jk)
                    V.tensor_add(ctv, ctv, ct2v)
            V.tensor_sub(gw4[:, :, :, 3:4], gw4[:, :, :, 3:4], ctv)

            if cfg["debug"]:
                nc.sync.dma_start(dbg["r9"].ap(), r9[:, :])
                nc.sync.dma_start(dbg["j"].ap(), j_sb[:, :])
                nc.sync.dma_start(dbg["gw"].ap(), gw[:, :])

            gat = statep.tile([NJ, 12 * B_LOC], CDT)
            gwe = gw[:, :].rearrange("p (j e) -> p e j", e=12)
            with tc.tile_pool(name="psT", bufs=3, space="PSUM") as psT:
                for e in range(12):
                    pgt = psT.tile([NJ, B_LOC], F32, tag="gt")
                    nc.tensor.transpose(pgt[:, :], gwe[:, e, :], ident[:, :])
                    V.tensor_copy(gat[:, e * B_LOC:(e + 1) * B_LOC], pgt[:, :])

            dirs_ap = dirs_d.ap().rearrange("k (c v) -> k c v", c=3)
            _main_loop_v2(nc, tc, cfg, ch, ODT, dirs_ap, coeffT_a, coeffT_b, gat,
                          wt_sb, out_d)

    nc.compile()
    _CACHE[key] = nc
    return nc


def _main_loop_v2(nc, tc, cfg, ch, ODT, dirs_ap, coeffT_a, coeffT_b, gat, wt_sb, out_d):
    V = nc.vector
    S = nc.scalar
    P = nc.gpsimd
    n_chunks = (NV + ch - 1) // ch
    with (
        tc.tile_pool(name="loop", bufs=3) as loopp,
        tc.tile_pool(name="psMM", bufs=2, space="PSUM") as psMM,
        tc.tile_pool(name="psTG", bufs=2, space="PSUM") as psTG,
    ):
        for ci in range(n_chunks):
            v0 = ci * ch
            sz = min(ch, NV - v0)
            da = loopp.tile([128, 3, ch], F16, tag="da")
            nc.sync.dma_start(da[:, :, 0:sz], dirs_ap[0:128, :, v0:v0 + sz])
            db = loopp.tile([90, 3, ch], F16, tag="db")
            nc.sync.dma_start(db[:, :, 0:sz], dirs_ap[128:KC, :, v0:v0 + sz])

            vp_sb = loopp.tile([B_LOC, 3, ch], F16, tag="vp")
            for c in range(3):
                pvc = psMM.tile([B_LOC, ch], F32, tag="mm")
                nc.tensor.matmul(pvc[:, 0:sz], coeffT_a[:, :], da[:, c, 0:sz],
                                 start=True, stop=False)
                nc.tensor.matmul(pvc[:, 0:sz], coeffT_b[:, :], db[:, c, 0:sz],
                                 start=False, stop=True)
                S.copy(vp_sb[:, c, 0:sz], pvc[:, 0:sz])

            t_sbs = []
            for n in range(3):
                ptn = psTG.tile([B_LOC, 3, ch], F32, tag="tg")
                for m in range(3):
                    e = m * 4 + n
                    nc.tensor.matmul(ptn[:, m, 0:sz],
                                     gat[:, e * B_LOC:(e + 1) * B_LOC],
                                     wt_sb[:, v0:v0 + sz], start=True, stop=True)
                t_sb = loopp.tile([B_LOC, 3, ch], F16, tag=f"tsb{n}")
                if sz == ch:
                    S.copy(t_sb[:, :, :], ptn[:, :, :])
                else:
                    for m in range(3):
                        S.copy(t_sb[:, m, 0:sz], ptn[:, m, 0:sz])
                t_sbs.append(t_sb)

            pt3 = psTG.tile([B_LOC, 3, ch], F32, tag="tg")
            for m in range(3):
                e = m * 4 + 3
                nc.tensor.matmul(pt3[:, m, 0:sz],
                                 gat[:, e * B_LOC:(e + 1) * B_LOC],
                                 wt_sb[:, v0:v0 + sz], start=True, stop=True)

            tmps = []
            for n in range(3):
                tmp = loopp.tile([B_LOC, 3, ch], F16, tag=f"tmp{n}")
                vb = vp_sb[:, n, 0:sz].unsqueeze(1).broadcast_to([B_LOC, 3, sz])
                eng = P if n == 2 else V
                eng.tensor_mul(tmp[:, :, 0:sz], t_sbs[n][:, :, 0:sz], vb)
                tmps.append(tmp)
            acc = loopp.tile([B_LOC, 3, ch], F16, tag="acc")
            V.tensor_add(acc[:, :, 0:sz], tmps[0][:, :, 0:sz], pt3[:, :, 0:sz])
            V.tensor_add(acc[:, :, 0:sz], acc[:, :, 0:sz], tmps[1][:, :, 0:sz])
            out_sb = loopp.tile([B_LOC, 3, ch], ODT, tag="outsb")
            V.tensor_add(out_sb[:, :, 0:sz], acc[:, :, 0:sz], tmps[2][:, :, 0:sz])
            nc.sync.dma_start(out_d.ap()[:, :, v0:v0 + sz], out_sb[:, :, 0:sz])


def _host_prep(inputs, cfg):
    fp16 = cfg["compute"] == "fp16"
    cdt = np.float16 if fp16 else np.float32
    shapedirs = np.asarray(inputs["shapedirs"], np.float32)
    posedirs = np.asarray(inputs["posedirs"], np.float32)
    v_template = np.asarray(inputs["v_template"], np.float32)
    Jreg = np.asarray(inputs["J_regressor"], np.float32)
    weights = np.asarray(inputs["weights"], np.float32)

    dirs = np.empty((KC, VC), np.float32)
    dirs[0:10] = shapedirs.transpose(2, 1, 0).reshape(10, VC)
    dirs[10:217] = posedirs.transpose(2, 1, 0).reshape(NP, VC)
    dirs[217] = v_template.T.reshape(VC)
    js2 = np.einsum('jv,vcs->sjc', Jreg, shapedirs).reshape(10, 72)
    jtmpl = (Jreg @ v_template).reshape(1, 72)
    return {
        "dirs": np.ascontiguousarray(dirs.astype(cdt)),
        "wt": np.ascontiguousarray(weights.T.astype(cdt)),
        "js2": np.ascontiguousarray(js2),
        "jtmpl": np.ascontiguousarray(jtmpl),
        "ident": np.eye(128, dtype=np.float32),
    }


# revision 52
# speedup vs baseline: 1.0411x; 1.0128x over previous
"""SMPL (shape blend + pose blend + LBS skinning) Bass kernel for 8 TRN2 NeuronCores.

Data-parallel over batch: B=1024 -> 128 per core. All SMPL buffers replicated.

Measured HW model this kernel is built around (NTFF traces on these cores):
  - PE runs at 1.2 GHz here (no HAM ramp observed), 1 psum column/cycle,
    out <= 512 fp32 cols per matmul (1 PSUM bank), ~300ns fixed cost per
    matmul + ~420ns LDWEIGHTS+gap, partially hidden by the queue.
  - fp8 DoubleRow streams 2 packed columns/cycle -> same out-column rate as
    fp16, but doubles K capacity per pass: used for the K=230 vp matmul
    (one mm instead of two per c-plane).
  - ACTIVATE (ScalarE) is 1x, ~(N+352)/1.2 ns; DVE fp16 tensor_tensor is 2x;
    a DVE op overlapping any GpSimd op drops to ~0.5x (shared SBUF port), so
    GpSimd is left idle on purpose.

Numerics: vp in fp8e4 DoubleRow with power-of-2 row scaling and a hi/lo
compensation split for v_template; skinning matmul T in fp16.
Measured rel err 7.7e-3 vs the 2e-2 gate.
  vp (K=220 packed into 110 rows x 2 panels):
      rows = [beta/16 x shapedirs*16 | lrot/16 x posedirs*16 |
              1 x tmpl_hi | 1/16 x tmpl_lo*16 | pad]
  (110-row DMA tiles are ~2x faster per byte than 115-row ones - the DMA
  rate is sharply sensitive to partition-row count; dirs loads are issued
  at chunk-PAIR granularity, 6KB per partition row.)

Phase structure (per core):
  prologue: ACT-table warmup at t=0; Rodrigues (V+S) -> scaled coeff ->
            fp32 transposes -> coeffT8 (fp8); J matmul; FK on V
  P1 loop (overlaps FK on V): 3 DR matmuls/chunk -> 1-bank vp psum tiles ->
            per-plane S copies into persistent vp_sb [128,3,7168] fp16. The
            12 gat transposes are emitted mid-P1 (TSPLIT=12) so the PE
            reaches them right as FK finishes; remaining P1 chunks keep the
            PE queue fed.
  P2 loop:  12 fp16 matmuls/chunk into 3-bank psum n-groups (bufs=2) +
            pt3 group; S evicts all 4 groups to fp16 (incl pt3 - frees the
            psum slot fast and keeps V in 2x mode); V: 3 broadcast muls +
            3-add chain (never in-place: dst==src DVE ops run 4x slower).
Last chunk runs at its true 234-col width. Output [14, 128, 3, 512] fp16
chunk-major; host reassembles to [1024, 6890, 3] fp32.
"""

import sys
import numpy as np
import ml_dtypes

for _p in ("/opt/trn_rl_repo",):
    if _p not in sys.path:
        sys.path.append(_p)

import concourse.bass as bass
import concourse.tile as tile
import concourse.mybir as mybir
from concourse import bacc
from concourse.bass_utils import run_bass_kernel_spmd
from concourse.alu_op_type import AluOpType

F32 = mybir.dt.float32
F16 = mybir.dt.float16
F8 = mybir.dt.float8e4
F8NP = ml_dtypes.float8_e4m3
DR = mybir.MatmulPerfMode.DoubleRow

N_CORES = 8
B = 1024
B_LOC = B // N_CORES  # 128
NV = 6890
NVP = 7168            # padded to 14*512
NCH = 14
CH = 512
NJ = 24
NP = 207

KVP = 220             # vp contraction (padded even): 10+207+2+1
KVH = KVP // 2        # 110
KT = 64               # T contraction rows (x2 panels = 128 logical);
                      # blocks at 32-aligned bases: hi @0, lo8 @32, hi16 @0/p1

# FK level groups: (child_start, n_children, parent_start, parent_broadcast)
FK_GROUPS = [
    (1, 3, 0, True),
    (4, 3, 1, False),
    (7, 3, 4, False),
    (10, 3, 7, False),
    (13, 2, 9, True),
    (15, 3, 12, False),
    (18, 2, 16, False),
    (20, 2, 18, False),
    (22, 2, 20, False),
]

CFG = {
    "compute": "fp8",    # "fp8" | "fp16" (legacy)
    "ch": 512,
    "out16": True,
    "vp_fp8": False,
    "debug": False,
    "trace": False,
}

_CACHE = {}


def _rodrigues_and_coeff(nc, tc, constp, statep, scrp, pose_sb, beta_sb):
    """Rodrigues rotation build (fp32) + scaled coeff [128, KVP] fp32.

    coeff rows: 0..9 beta/16, 10..216 lrot/16, 217 = 1.0, 218 = 1/16,
    219 = 0.
    Returns (r9, coeff, beta_sb)."""
    V = nc.vector
    S = nc.scalar

    # dummy activation at t=0: pulls the Sqrt ACT-table load off the
    # Rodrigues critical path (it overlaps the pose DMA instead). Only the
    # Sqrt set is warmed: warming Sin too would thrash (the table RAM holds
    # one set at a time, so Sqrt would reload on the path; measured 4 loads
    # instead of 2).
    warm = scrp.tile([B_LOC, 1], F32, tag="warm")
    V.memset(warm[:, :], 1.0)
    S.activation(warm[:, :], warm[:, :], mybir.ActivationFunctionType.Sqrt)

    sq = scrp.tile([B_LOC, 72], F32, tag="sq")
    V.tensor_mul(sq[:, :], pose_sb[:, :], pose_sb[:, :])
    sq3 = sq[:, :].rearrange("p (j c) -> p c j", c=3)
    th2 = scrp.tile([B_LOC, NJ], F32, tag="th2")
    V.tensor_add(th2[:, :], sq3[:, 0, :], sq3[:, 1, :])
    V.tensor_add(th2[:, :], th2[:, :], sq3[:, 2, :])
    cbias = constp.tile([128, 2], F32)
    V.memset(cbias[:, 0:1], 1e-8)
    V.memset(cbias[:, 1:2], float(np.pi / 2))
    theta = scrp.tile([B_LOC, NJ], F32, tag="theta")
    S.activation(theta[:, :], th2[:, :], mybir.ActivationFunctionType.Sqrt,
                 bias=cbias[0:B_LOC, 0:1])
    invt = scrp.tile([B_LOC, NJ], F32, tag="invt")
    V.reciprocal(invt[:, :], theta[:, :])
    sh = scrp.tile([B_LOC, NJ], F32, tag="sh")
    S.activation(sh[:, :], theta[:, :], mybir.ActivationFunctionType.Sin, scale=0.5)
    chh = scrp.tile([B_LOC, NJ], F32, tag="chh")
    S.activation(chh[:, :], theta[:, :], mybir.ActivationFunctionType.Sin,
                 scale=0.5, bias=cbias[0:B_LOC, 1:2])
    s_t = scrp.tile([B_LOC, NJ], F32, tag="s_t")
    V.scalar_tensor_tensor(s_t[:, :], sh[:, :], 2.0, chh[:, :], AluOpType.mult, AluOpType.mult)
    shsq = scrp.tile([B_LOC, NJ], F32, tag="shsq")
    V.tensor_mul(shsq[:, :], sh[:, :], sh[:, :])
    c_t = scrp.tile([B_LOC, NJ], F32, tag="c_t")
    V.tensor_scalar(c_t[:, :], shsq[:, :], -2.0, 1.0, AluOpType.mult, AluOpType.add)
    omc = scrp.tile([B_LOC, NJ], F32, tag="omc")
    V.tensor_scalar_mul(omc[:, :], shsq[:, :], 2.0)
    ax = scrp.tile([B_LOC, 72], F32, tag="ax")
    ax3 = ax[:, :].rearrange("p (j c) -> p c j", c=3)
    p3 = pose_sb[:, :].rearrange("p (j c) -> p c j", c=3)
    V.tensor_mul(ax3[:, :, :], p3[:, :, :],
                 invt[:, :].unsqueeze(1).broadcast_to([B_LOC, 3, NJ]))
    # batched outer products: [xx yy zz xy xz yz sx sy sz] in 5 ops instead
    # of 15 (each ~180ns of serial V time on the coeffT8 critical chain)
    pr9 = scrp.tile([B_LOC, 9, NJ], F32, tag="pr9")
    prm = scrp.tile([B_LOC, 6, NJ], F32, tag="prm")
    V.tensor_mul(pr9[:, 0:3, :], ax3[:, 0:3, :], ax3[:, 0:3, :])
    V.tensor_mul(pr9[:, 3:5, :],
                 ax3[:, 0:1, :].broadcast_to([B_LOC, 2, NJ]), ax3[:, 1:3, :])
    V.tensor_mul(pr9[:, 5:6, :], ax3[:, 1:2, :], ax3[:, 2:3, :])
    V.tensor_mul(prm[:, :, :], pr9[:, 0:6, :],
                 omc[:, :].unsqueeze(1).broadcast_to([B_LOC, 6, NJ]))
    V.tensor_mul(pr9[:, 6:9, :],
                 s_t[:, :].unsqueeze(1).broadcast_to([B_LOC, 3, NJ]),
                 ax3[:, 0:3, :])
    prods = {"xx": prm[:, 0, :], "yy": prm[:, 1, :], "zz": prm[:, 2, :],
             "xy": prm[:, 3, :], "xz": prm[:, 4, :], "yz": prm[:, 5, :],
             "sx": pr9[:, 6, :], "sy": pr9[:, 7, :], "sz": pr9[:, 8, :]}
    r9 = statep.tile([B_LOC, NJ * 9], F32)
    r9e = r9[:, :].rearrange("p (j e) -> p e j", e=9)
    ENTRIES = [
        ("add", "c", "xx"), ("sub", "xy", "sz"), ("add", "xz", "sy"),
        ("add", "xy", "sz"), ("add", "c", "yy"), ("sub", "yz", "sx"),
        ("sub", "xz", "sy"), ("add", "yz", "sx"), ("add", "c", "zz"),
    ]
    for e, (op, a, b_) in enumerate(ENTRIES):
        ta = c_t[:, :] if a == "c" else prods[a]
        fn = V.tensor_add if op == "add" else V.tensor_sub
        fn(r9e[:, e, :], ta, prods[b_])

    # ---- scaled coeff ----
    coeff = statep.tile([B_LOC, KVP], F32)
    V.tensor_scalar_mul(coeff[:, 0:10], beta_sb[:, :], 1.0 / 16.0)
    # lrot/16 with diag -1/16
    V.tensor_scalar_mul(coeff[:, 10:217], r9[:, 9:216], 1.0 / 16.0)
    lr9 = coeff[:, 10:217].rearrange("p (j e) -> p e j", e=9)
    for e in (0, 4, 8):
        V.tensor_scalar_add(lr9[:, e, :], lr9[:, e, :], -1.0 / 16.0)
    V.memset(coeff[:, 217:218], 1.0)
    V.memset(coeff[:, 218:219], 1.0 / 16.0)
    V.memset(coeff[:, 219:220], 0.0)
    return r9, coeff, beta_sb


def build_program_fp8(cfg):
    key = ("fp8", cfg["debug"])
    if key in _CACHE:
        return _CACHE[key]

    nc = bacc.Bacc("TRN2", target_bir_lowering=False, debug=False)

    pose_d = nc.dram_tensor("pose", [B_LOC, 72], F32, kind="ExternalInput")
    beta_d = nc.dram_tensor("beta", [B_LOC, 10], F32, kind="ExternalInput")
    # chunk-PAIR granularity: 6144B per partition row per DMA (115 x 3KB
    # descriptors measured only ~23GB/s/engine; P1 was DMA-bandwidth bound
    # at 2.85us/chunk)
    dirs8_d = nc.dram_tensor("dirs8", [NCH // 2, KVH, 2, 2, 3, CH], F8,
                             kind="ExternalInput")
    # wt replicated at partition bases 0/32/64: T matmuls rotate across
    # three 32-row PE quadrants so LDWEIGHTS can be pulled ahead into idle
    # rows while the previous matmul streams (re-validated: the earlier
    # "regression" was device throttling; fast-equivalent T-mm is ~600 vs
    # 625ns)
    wt16_d = nc.dram_tensor("wt16", [NCH, 128, CH], F16, kind="ExternalInput")
    js2_d = nc.dram_tensor("js2", [10, 72], F32, kind="ExternalInput")
    jtmpl_d = nc.dram_tensor("jtmpl", [1, 72], F32, kind="ExternalInput")
    ident_d = nc.dram_tensor("ident", [128, 128], F32, kind="ExternalInput")
    out_d = nc.dram_tensor("out", [NCH, B_LOC, 3, CH], F16, kind="ExternalOutput")
    dbg = {}
    if cfg["debug"]:
        dbg["r9"] = nc.dram_tensor("dbg_r9", [B_LOC, 216], F32, kind="ExternalOutput")
        dbg["j"] = nc.dram_tensor("dbg_j", [B_LOC, 72], F32, kind="ExternalOutput")
        dbg["gw"] = nc.dram_tensor("dbg_gw", [B_LOC, 288], F32, kind="ExternalOutput")
        dbg["vp"] = nc.dram_tensor("dbg_vp", [B_LOC, 3, NVP], F16, kind="ExternalOutput")

    with tile.TileContext(nc) as tc:
        with (
            tc.tile_pool(name="const", bufs=1) as constp,
            tc.tile_pool(name="state", bufs=1) as statep,
            tc.tile_pool(name="scr", bufs=1) as scrp,
        ):
            V = nc.vector
            S = nc.scalar
            # pose/beta first on the sync queue: they gate the whole
            # Rodrigues critical chain (ident/js2/jtmpl aren't needed until
            # the transposes ~10us later)
            pose_sb = statep.tile([B_LOC, 72], F32)
            nc.sync.dma_start(pose_sb[:, :], pose_d.ap())
            beta_sb = statep.tile([B_LOC, 10], F32)
            nc.sync.dma_start(beta_sb[:, :], beta_d.ap())
            ident = constp.tile([128, 128], F32)
            nc.sync.dma_start(ident[:, :], ident_d.ap())
            js2 = statep.tile([10, 72], F32)
            nc.sync.dma_start(js2[:, :], js2_d.ap())
            jtmpl = statep.tile([1, 72], F32)
            nc.sync.dma_start(jtmpl[:, :], jtmpl_d.ap())

            r9, coeff, beta_sb = _rodrigues_and_coeff(
                nc, tc, constp, statep, scrp, pose_sb, beta_sb)

            # ---- coeffT8 [115, 2, 128] via two fp32 transposes + fp8 cast ----
            coeffT8 = statep.tile([KVH, 2, B_LOC], F8)
            betaT = statep.tile([10, B_LOC], F32)
            j_sb = statep.tile([B_LOC, 72], F32)
            with tc.tile_pool(name="psA", bufs=2, space="PSUM") as psA:
                ptA = psA.tile([KVH, 128], F32, tag="tp")
                nc.tensor.transpose(ptA[:, :], coeff[:, 0:KVH], ident[:, :])
                V.tensor_copy(coeffT8[:, 0, :], ptA[:, 0:B_LOC])
                V.tensor_scalar_mul(betaT[:, :], ptA[0:10, 0:B_LOC], 16.0)
                ptB = psA.tile([KVH, 128], F32, tag="tp")
                nc.tensor.transpose(ptB[:, :], coeff[:, KVH:KVP], ident[:, :])
                V.tensor_copy(coeffT8[:, 1, :], ptB[:, 0:B_LOC])

                # ---- J = [beta | 1] @ [JS2 ; Jtmpl] ----
                pj = psA.tile([B_LOC, 72], F32, tag="pj")
                onesT = statep.tile([1, B_LOC], F32)
                V.memset(onesT[0:1, :], 1.0)
                nc.tensor.matmul(pj[:, :], betaT[:, :], js2[:, :], start=True, stop=False)
                nc.tensor.matmul(pj[:, :], onesT[0:1, :], jtmpl[0:1, :], start=False, stop=True)
                V.tensor_copy(j_sb[:, :], pj[:, :])

            # ---- P1: vp matmuls (independent of FK; emitted before gat
            # transposes so the PE queue is not blocked behind FK).
            # _p1_chunk is invoked for chunks 0..TSPLIT-1 here and the rest
            # after the gat transposes, so the transposes (which wait on FK)
            # slot into the PE queue right when FK finishes. ----
            vp_sb = statep.tile([B_LOC, 3, NVP], F16)
            p1_ctx = ctx = tc.tile_pool(name="p1", bufs=4)
            p1p = ctx.__enter__()
            # wide 3-bank vp psum tiles with ONE wide S copy per chunk:
            # ScalarE is the global wall (86% busy), so minimizing ACT
            # per-op overhead beats finer psum-release granularity
            psVP_ctx = tc.tile_pool(name="psVP", bufs=2, space="PSUM")
            psVP = psVP_ctx.__enter__()

            da8_pair = [None]

            def _p1_chunk(ci):
                sz = min(CH, NV - ci * CH)
                if ci % 2 == 0:
                    da8 = p1p.tile([KVH, 2, 2, 3, CH], F8, tag="da")
                    da8_pair[0] = da8
                    nc.sync.dma_start(da8[:, :, :, :, :],
                                      dirs8_d.ap()[ci // 2])
                da8 = da8_pair[0]
                pvc = psVP.tile([B_LOC, 3, CH], F32, tag="vp")
                for c in range(3):
                    nc.tensor.matmul(pvc[:, c, 0:sz], coeffT8[:, :, :],
                                     da8[:, ci % 2, :, c, 0:sz], start=True,
                                     stop=True, perf_mode=DR)
                S.copy(vp_sb[:, :, ci * CH:ci * CH + sz], pvc[:, :, 0:sz])

            TSPLIT = 12
            for ci in range(TSPLIT):
                _p1_chunk(ci)

            # ---- J_rel ----
            jrel = statep.tile([B_LOC, 72], F32)
            jv = j_sb[:, :].rearrange("p (j c) -> p j c", c=3)
            jrv = jrel[:, :].rearrange("p (j c) -> p j c", c=3)
            V.tensor_copy(jrel[:, 0:3], j_sb[:, 0:3])
            V.tensor_sub(jrv[:, 1:4], jv[:, 1:4], jv[:, 0:1].broadcast_to([B_LOC, 3, 3]))
            V.tensor_sub(jrv[:, 4:12], jv[:, 4:12], jv[:, 1:9])
            V.tensor_sub(jrv[:, 12:15], jv[:, 12:15], jv[:, 9:10].broadcast_to([B_LOC, 3, 3]))
            V.tensor_sub(jrv[:, 15:18], jv[:, 15:18], jv[:, 12:15])
            V.tensor_sub(jrv[:, 18:24], jv[:, 18:24], jv[:, 16:22])

            # ---- local transforms Gl [128, 24*12] (3x4 row-major [R|t]) ----
            gl = statep.tile([B_LOC, NJ * 12], F32)
            gl4 = gl[:, :].rearrange("p (j m n) -> p j m n", m=3, n=4)
            r94 = r9[:, :].rearrange("p (j m n) -> p j m n", m=3, n=3)
            V.tensor_copy(gl4[:, :, :, 0:3], r94[:, :, :, :])
            V.tensor_copy(gl4[:, :, :, 3:4], jrv[:, :, :].unsqueeze(3))

            # ---- forward kinematics ----
            gw = statep.tile([B_LOC, NJ * 12], F32)
            gw4 = gw[:, :].rearrange("p (j m n) -> p j m n", m=3, n=4)
            V.tensor_copy(gw[:, 0:12], gl[:, 0:12])
            fktmp = scrp.tile([B_LOC, 3 * 12], F32, tag="fktmp")
            for (c0, ncld, p0, bc) in FK_GROUPS:
                child = gw4[:, c0:c0 + ncld]
                loc = gl4[:, c0:c0 + ncld]
                par = gw4[:, p0:p0 + (1 if bc else ncld)]
                tmpv = fktmp[:, 0:ncld * 12].rearrange("p (j m n) -> p j m n", m=3, n=4)
                shp = [B_LOC, ncld, 3, 4]
                for k in range(3):
                    in0 = loc[:, :, k:k + 1, :].broadcast_to(shp)
                    pk = par[:, 0:1, :, k:k + 1] if bc else par[:, :, :, k:k + 1]
                    in1 = pk.broadcast_to(shp)
                    if k == 0:
                        V.tensor_mul(child[:, :, :, :], in0, in1)
                    else:
                        V.tensor_mul(tmpv, in0, in1)
                        V.tensor_add(child[:, :, :, :], child[:, :, :, :], tmpv)
                ptr = par[:, 0:1, :, 3:4] if bc else par[:, :, :, 3:4]
                V.tensor_add(child[:, :, :, 3:4], child[:, :, :, 3:4],
                             ptr.broadcast_to([B_LOC, ncld, 3, 1]))

            # ---- rest-pose correction: t_j -= R_j^w @ J_j ----
            ct = scrp.tile([B_LOC, 72], F32, tag="ct")
            ct2 = scrp.tile([B_LOC, 72], F32, tag="ct2")
            ctv = ct[:, :].rearrange("p (j m) -> p j m", m=3).unsqueeze(3)
            ct2v = ct2[:, :].rearrange("p (j m) -> p j m", m=3).unsqueeze(3)
            for k in range(3):
                jk = jv[:, :, k:k + 1].unsqueeze(2).broadcast_to([B_LOC, NJ, 3, 1])
                if k == 0:
                    V.tensor_mul(ctv, gw4[:, :, :, k:k + 1], jk)
                else:
                    V.tensor_mul(ct2v, gw4[:, :, :, k:k + 1], jk)
                    V.tensor_add(ctv, ctv, ct2v)
            V.tensor_sub(gw4[:, :, :, 3:4], gw4[:, :, :, 3:4], ctv)

            if cfg["debug"]:
                nc.sync.dma_start(dbg["r9"].ap(), r9[:, :])
                nc.sync.dma_start(dbg["j"].ap(), j_sb[:, :])
                nc.sync.dma_start(dbg["gw"].ap(), gw[:, :])

            # ---- gat16 via 12 fp32 transposes: [24, 12, 128] fp16 ----
            gat16 = statep.tile([NJ, 12, B_LOC], F16)
            gwe = gw[:, :].rearrange("p (j e) -> p e j", e=12)
            with tc.tile_pool(name="psT", bufs=2, space="PSUM") as psT:
                for e in range(12):
                    pgt = psT.tile([NJ, B_LOC], F32, tag="gt")
                    nc.tensor.transpose(pgt[:, :], gwe[:, e, :], ident[:, :])
                    V.tensor_copy(gat16[:, e, :], pgt[:, :])
            # replicate gat at the three quadrant bases (base 96 is rejected
            # by bass) for the quadrant-rotated T matmuls
            gat16x = statep.tile([96, 12, B_LOC], F16)
            for q in range(3):
                nc.sync.dma_start(gat16x[32 * q:32 * q + NJ, :, :],
                                  gat16[:, :, :])
            # remaining P1 chunks fill the PE queue behind the transposes
            for ci in range(TSPLIT, NCH):
                _p1_chunk(ci)
            psVP_ctx.__exit__(None, None, None)
            p1_ctx.__exit__(None, None, None)

            if cfg["debug"]:
                nc.sync.dma_start(dbg["vp"].ap(), vp_sb[:, :, :])

            # ---- P2: skinning matmuls + combine ----
            with (
                tc.tile_pool(name="p2", bufs=3) as p2p,
                tc.tile_pool(name="psTG", bufs=2, space="PSUM") as psTG,
            ):
                qi = 0
                for ci in range(NCH):
                    v0 = ci * CH
                    sz = min(CH, NV - v0)
                    wt16c = p2p.tile([128, CH], F16, tag="wt")
                    nc.sync.dma_start(wt16c[:, :], wt16_d.ap()[ci])

                    t_sbs = []
                    for n in range(3):
                        ptn = psTG.tile([B_LOC, 3, CH], F32, tag="tg")
                        for m in range(3):
                            e = m * 4 + n
                            q = qi % 3
                            qi += 1
                            nc.tensor.matmul(
                                ptn[:, m, 0:sz],
                                gat16x[32 * q:32 * q + NJ, e, :],
                                wt16c[32 * q:32 * q + NJ, 0:sz],
                                start=True, stop=True)
                        t_sb = p2p.tile([B_LOC, 3, CH], F16, tag=f"tsb{n}")
                        S.copy(t_sb[:, :, 0:sz], ptn[:, :, 0:sz])
                        t_sbs.append(t_sb)

                    pt3 = psTG.tile([B_LOC, 3, CH], F32, tag="tg")
                    for m in range(3):
                        e = m * 4 + 3
                        q = qi % 3
                        qi += 1
                        nc.tensor.matmul(pt3[:, m, 0:sz],
                                         gat16x[32 * q:32 * q + NJ, e, :],
                                         wt16c[32 * q:32 * q + NJ, 0:sz],
                                         start=True, stop=True)
                    # pt3 evicted by S too: frees its psum slot fast (PE would
                    # otherwise stall on the rotation) and keeps the V add in
                    # fp16 2x mode. GpSimd is NOT used: it shares the DVE SBUF
                    # port, halving any concurrent 2-port V op (measured
                    # 950ns -> 3200ns).
                    pt3_sb = p2p.tile([B_LOC, 3, CH], F16, tag="pt3sb")
                    if ci in (4, 9):
                        # level S (99.5% busy) vs V (90.5%): V takes 2 of the
                        # 14 pt3 evictions
                        V.tensor_copy(pt3_sb[:, :, 0:sz], pt3[:, :, 0:sz])
                    else:
                        S.copy(pt3_sb[:, :, 0:sz], pt3[:, :, 0:sz])

                    tmps = []
                    for n in range(3):
                        tmp = p2p.tile([B_LOC, 3, CH], F16, tag=f"tmp{n}")
                        vb = vp_sb[:, n, v0:v0 + sz].unsqueeze(1).broadcast_to(
                            [B_LOC, 3, sz])
                        V.tensor_mul(tmp[:, :, 0:sz], t_sbs[n][:, :, 0:sz], vb)
                        tmps.append(tmp)
                    acc = p2p.tile([B_LOC, 3, CH], F16, tag="acc")
                    V.tensor_add(acc[:, :, 0:sz], tmps[0][:, :, 0:sz],
                                 pt3_sb[:, :, 0:sz])
                    acc2 = p2p.tile([B_LOC, 3, CH], F16, tag="acc2")
                    V.tensor_add(acc2[:, :, 0:sz], acc[:, :, 0:sz],
                                 tmps[1][:, :, 0:sz])
                    out_sb = p2p.tile([B_LOC, 3, CH], F16, tag="outsb")
                    V.tensor_add(out_sb[:, :, 0:sz], acc2[:, :, 0:sz],
                                 tmps[2][:, :, 0:sz])
                    nc.sync.dma_start(out_d.ap()[ci][:, :, 0:sz],
                                      out_sb[:, :, 0:sz])

    nc.compile()
    _CACHE[key] = nc
    return nc


def _host_prep_fp8(inputs):
    f32 = np.float32
    shapedirs = np.asarray(inputs["shapedirs"], f32)   # [V,3,10]
    posedirs = np.asarray(inputs["posedirs"], f32)     # [V,3,207]
    v_template = np.asarray(inputs["v_template"], f32)  # [V,3]
    Jreg = np.asarray(inputs["J_regressor"], f32)       # [24,V]
    weights = np.asarray(inputs["weights"], f32)        # [V,24]

    dirs = np.zeros((KVP, 3, NVP), f32)
    sd = shapedirs.transpose(2, 1, 0)   # [10,3,V]
    pd = posedirs.transpose(2, 1, 0)    # [207,3,V]
    dirs[0:10, :, :NV] = sd * 16.0
    dirs[10:217, :, :NV] = pd * 16.0
    tmpl = v_template.T
    hi8 = tmpl.astype(F8NP).astype(f32)
    dirs[217, :, :NV] = hi8
    dirs[218, :, :NV] = (tmpl - hi8) * 16.0
    dirs8 = dirs.astype(F8NP)
    d = dirs8.reshape(KVP, 3, NCH, CH)
    dirs8_arr = np.empty((NCH, KVH, 2, 3, CH), F8NP)
    dirs8_arr[:, :, 0] = d[0:KVH].transpose(2, 0, 1, 3)
    dirs8_arr[:, :, 1] = d[KVH:KVP].transpose(2, 0, 1, 3)
    # [NCH,...] -> chunk-pair-major [NCH/2, KVH, 2(chunk), 2(panel), 3, CH]
    dirs8_arr = dirs8_arr.reshape(NCH // 2, 2, KVH, 2, 3, CH).transpose(
        0, 2, 1, 3, 4, 5)

    wt = np.zeros((NJ, NVP), np.float16)
    wt[:, :NV] = weights.T.astype(np.float16)
    wtc = wt.reshape(NJ, NCH, CH).transpose(1, 0, 2)    # [NCH, NJ, CH]
    wt16_arr = np.zeros((NCH, 128, CH), np.float16)
    for q in range(3):
        wt16_arr[:, 32 * q:32 * q + NJ] = wtc
    wt16_arr = np.ascontiguousarray(wt16_arr)

    js2 = np.einsum('jv,vcs->sjc', Jreg, shapedirs).reshape(10, 72)
    jtmpl = (Jreg @ v_template).reshape(1, 72)
    return {
        "dirs8": np.ascontiguousarray(dirs8_arr),
        "wt16": wt16_arr,
        "js2": np.ascontiguousarray(js2),
        "jtmpl": np.ascontiguousarray(jtmpl),
        "ident": np.eye(128, dtype=f32),
    }


def kernel(pose, beta, shapedirs, posedirs, v_template, J_regressor, weights):
    cfg = CFG
    if cfg["compute"] == "fp8":
        nc = build_program_fp8(cfg)
        rep = _host_prep_fp8(dict(shapedirs=shapedirs, posedirs=posedirs,
                                  v_template=v_template, J_regressor=J_regressor,
                                  weights=weights))
    else:
        nc = build_program(cfg)
        rep = _host_prep(dict(shapedirs=shapedirs, posedirs=posedirs,
                              v_template=v_template, J_regressor=J_regressor,
                              weights=weights), cfg)
    pose = np.asarray(pose, np.float32)
    beta = np.asarray(beta, np.float32)
    in_maps = []
    for i in range(N_CORES):
        m = dict(rep)
        m["pose"] = np.ascontiguousarray(pose[i * B_LOC:(i + 1) * B_LOC])
        m["beta"] = np.ascontiguousarray(beta[i * B_LOC:(i + 1) * B_LOC])
        in_maps.append(m)
    res = run_bass_kernel_spmd(nc, in_maps, core_ids=list(range(N_CORES)),
                               trace=cfg.get("trace", False))
    kernel.last_results = res
    outs = []
    for i in range(N_CORES):
        o = np.asarray(res.results[i]["out"], np.float32)
        if cfg["compute"] == "fp8":
            # [NCH, 128, 3, CH] -> [128, 3, NVP] -> [128, NV, 3]
            o = o.transpose(1, 2, 0, 3).reshape(B_LOC, 3, NVP)[:, :, :NV]
        outs.append(o.transpose(0, 2, 1))
    return np.ascontiguousarray(np.concatenate(outs, axis=0))


# ---------------------------------------------------------------------------
# Legacy fp16 path (kept for A/B testing via CFG["compute"]="fp16")
# ---------------------------------------------------------------------------
KC = 218
VC = 3 * NV


def build_program(cfg):
    key = (cfg["compute"], cfg["ch"], cfg["out16"], cfg["debug"])
    if key in _CACHE:
        return _CACHE[key]

    fp16 = cfg["compute"] == "fp16"
    CDT = F16 if fp16 else F32
    ODT = F16 if (fp16 and cfg["out16"]) else F32
    ch = cfg["ch"] if fp16 else 256

    nc = bacc.Bacc("TRN2", target_bir_lowering=False, debug=False)

    pose_d = nc.dram_tensor("pose", [B_LOC, 72], F32, kind="ExternalInput")
    beta_d = nc.dram_tensor("beta", [B_LOC, 10], F32, kind="ExternalInput")
    dirs_d = nc.dram_tensor("dirs", [KC, VC], CDT, kind="ExternalInput")
    wt_d = nc.dram_tensor("wt", [NJ, NV], CDT, kind="ExternalInput")
    js2_d = nc.dram_tensor("js2", [10, 72], F32, kind="ExternalInput")
    jtmpl_d = nc.dram_tensor("jtmpl", [1, 72], F32, kind="ExternalInput")
    ident_d = nc.dram_tensor("ident", [128, 128], F32, kind="ExternalInput")
    out_d = nc.dram_tensor("out", [B_LOC, 3, NV], ODT, kind="ExternalOutput")
    dbg = {}
    if cfg["debug"]:
        dbg["r9"] = nc.dram_tensor("dbg_r9", [B_LOC, 216], F32, kind="ExternalOutput")
        dbg["j"] = nc.dram_tensor("dbg_j", [B_LOC, 72], F32, kind="ExternalOutput")
        dbg["gw"] = nc.dram_tensor("dbg_gw", [B_LOC, 288], F32, kind="ExternalOutput")
        dbg["vp"] = nc.dram_tensor("dbg_vp", [B_LOC, 3, NV], F32, kind="ExternalOutput")

    with tile.TileContext(nc) as tc:
        with (
            tc.tile_pool(name="const", bufs=1) as constp,
            tc.tile_pool(name="state", bufs=1) as statep,
            tc.tile_pool(name="scr", bufs=1) as scrp,
        ):
            ident = constp.tile([128, 128], F32)
            nc.sync.dma_start(ident[:, :], ident_d.ap())
            wt_sb = constp.tile([NJ, NV], CDT)
            nc.sync.dma_start(wt_sb[:, :], wt_d.ap())
            js2 = statep.tile([10, 72], F32)
            nc.sync.dma_start(js2[:, :], js2_d.ap())
            jtmpl = statep.tile([1, 72], F32)
            nc.sync.dma_start(jtmpl[:, :], jtmpl_d.ap())
            pose_sb = statep.tile([B_LOC, 72], F32)
            nc.sync.dma_start(pose_sb[:, :], pose_d.ap())

            V = nc.vector
            S = nc.scalar
            sq = scrp.tile([B_LOC, 72], F32, tag="sq")
            V.tensor_mul(sq[:, :], pose_sb[:, :], pose_sb[:, :])
            sq3 = sq[:, :].rearrange("p (j c) -> p c j", c=3)
            th2 = scrp.tile([B_LOC, NJ], F32, tag="th2")
            V.tensor_add(th2[:, :], sq3[:, 0, :], sq3[:, 1, :])
            V.tensor_add(th2[:, :], th2[:, :], sq3[:, 2, :])
            cbias = constp.tile([128, 2], F32)
            V.memset(cbias[:, 0:1], 1e-8)
            V.memset(cbias[:, 1:2], float(np.pi / 2))
            theta = scrp.tile([B_LOC, NJ], F32, tag="theta")
            S.activation(theta[:, :], th2[:, :], mybir.ActivationFunctionType.Sqrt,
                         bias=cbias[0:B_LOC, 0:1])
            invt = scrp.tile([B_LOC, NJ], F32, tag="invt")
            V.reciprocal(invt[:, :], theta[:, :])
            sh = scrp.tile([B_LOC, NJ], F32, tag="sh")
            S.activation(sh[:, :], theta[:, :], mybir.ActivationFunctionType.Sin, scale=0.5)
            chh = scrp.tile([B_LOC, NJ], F32, tag="chh")
            S.activation(chh[:, :], theta[:, :], mybir.ActivationFunctionType.Sin,
                         scale=0.5, bias=cbias[0:B_LOC, 1:2])
            s_t = scrp.tile([B_LOC, NJ], F32, tag="s_t")
            V.scalar_tensor_tensor(s_t[:, :], sh[:, :], 2.0, chh[:, :], AluOpType.mult, AluOpType.mult)
            shsq = scrp.tile([B_LOC, NJ], F32, tag="shsq")
            V.tensor_mul(shsq[:, :], sh[:, :], sh[:, :])
            c_t = scrp.tile([B_LOC, NJ], F32, tag="c_t")
            V.tensor_scalar(c_t[:, :], shsq[:, :], -2.0, 1.0, AluOpType.mult, AluOpType.add)
            omc = scrp.tile([B_LOC, NJ], F32, tag="omc")
            V.tensor_scalar_mul(omc[:, :], shsq[:, :], 2.0)
            ax = scrp.tile([B_LOC, 72], F32, tag="ax")
            ax3 = ax[:, :].rearrange("p (j c) -> p c j", c=3)
            p3 = pose_sb[:, :].rearrange("p (j c) -> p c j", c=3)
            for ci in range(3):
                V.tensor_mul(ax3[:, ci, :], p3[:, ci, :], invt[:, :])
            prods = {}
            for name, (a, b_) in {
                "xx": (0, 0), "yy": (1, 1), "zz": (2, 2),
                "xy": (0, 1), "xz": (0, 2), "yz": (1, 2),
            }.items():
                t = scrp.tile([B_LOC, NJ], F32, tag="prod_" + name)
                V.tensor_mul(t[:, :], ax3[:, a, :], ax3[:, b_, :])
                V.tensor_mul(t[:, :], t[:, :], omc[:, :])
                prods[name] = t
            for name, a in {"sx": 0, "sy": 1, "sz": 2}.items():
                t = scrp.tile([B_LOC, NJ], F32, tag="prod_" + name)
                V.tensor_mul(t[:, :], s_t[:, :], ax3[:, a, :])
                prods[name] = t
            r9 = statep.tile([B_LOC, NJ * 9], F32)
            r9e = r9[:, :].rearrange("p (j e) -> p e j", e=9)
            ENTRIES = [
                ("add", "c", "xx"), ("sub", "xy", "sz"), ("add", "xz", "sy"),
                ("add", "xy", "sz"), ("add", "c", "yy"), ("sub", "yz", "sx"),
                ("sub", "xz", "sy"), ("add", "yz", "sx"), ("add", "c", "zz"),
            ]
            for e, (op, a, b_) in enumerate(ENTRIES):
                ta = c_t if a == "c" else prods[a]
                fn = V.tensor_add if op == "add" else V.tensor_sub
                fn(r9e[:, e, :], ta[:, :], prods[b_][:, :])

            coeff = statep.tile([B_LOC, KC], F32)
            nc.sync.dma_start(coeff[:, 0:10], beta_d.ap())
            V.tensor_copy(coeff[:, 10:217], r9[:, 9:216])
            lr9 = coeff[:, 10:217].rearrange("p (j e) -> p e j", e=9)
            for e in (0, 4, 8):
                V.tensor_scalar_add(lr9[:, e, :], lr9[:, e, :], -1.0)
            V.memset(coeff[:, 217:218], 1.0)

            with tc.tile_pool(name="psA", bufs=2, space="PSUM") as psA:
                pt1 = psA.tile([128, 128], F32, tag="tp")
                nc.tensor.transpose(pt1[:, :], coeff[:, 0:128], ident[:, :])
                coeffT_a = statep.tile([128, B_LOC], CDT)
                V.tensor_copy(coeffT_a[:, :], pt1[:, :])
                pt2 = psA.tile([128, 128], F32, tag="tp")
                nc.tensor.transpose(pt2[0:90, :], coeff[:, 128:218], ident[:, :])
                coeffT_b = statep.tile([90, B_LOC], CDT)
                V.tensor_copy(coeffT_b[:, :], pt2[0:90, :])

                pj = psA.tile([B_LOC, 72], F32, tag="pj")
                onesT = statep.tile([1, B_LOC], F32)
                V.memset(onesT[0:1, :], 1.0)
                if fp16:
                    betaT = statep.tile([10, B_LOC], F32)
                    V.tensor_copy(betaT[:, :], pt1[0:10, :])
                    betaT_ap = betaT[:, :]
                else:
                    betaT_ap = coeffT_a[0:10, :]
                nc.tensor.matmul(pj[:, :], betaT_ap, js2[:, :], start=True, stop=False)
                nc.tensor.matmul(pj[:, :], onesT[0:1, :], jtmpl[0:1, :], start=False, stop=True)
                j_sb = statep.tile([B_LOC, 72], F32)
                V.tensor_copy(j_sb[:, :], pj[:, :])

            jrel = statep.tile([B_LOC, 72], F32)
            jv = j_sb[:, :].rearrange("p (j c) -> p j c", c=3)
            jrv = jrel[:, :].rearrange("p (j c) -> p j c", c=3)
            V.tensor_copy(jrel[:, 0:3], j_sb[:, 0:3])
            V.tensor_sub(jrv[:, 1:4], jv[:, 1:4], jv[:, 0:1].broadcast_to([B_LOC, 3, 3]))
            V.tensor_sub(jrv[:, 4:12], jv[:, 4:12], jv[:, 1:9])
            V.tensor_sub(jrv[:, 12:15], jv[:, 12:15], jv[:, 9:10].broadcast_to([B_LOC, 3, 3]))
            V.tensor_sub(jrv[:, 15:18], jv[:, 15:18], jv[:, 12:15])
            V.tensor_sub(jrv[:, 18:24], jv[:, 18:24], jv[:, 16:22])

            gl = statep.tile([B_LOC, NJ * 12], F32)
            gl4 = gl[:, :].rearrange("p (j m n) -> p j m n", m=3, n=4)
            r94 = r9[:, :].rearrange("p (j m n) -> p j m n", m=3, n=3)
            V.tensor_copy(gl4[:, :, :, 0:3], r94[:, :, :, :])
            V.tensor_copy(gl4[:, :, :, 3:4], jrv[:, :, :].unsqueeze(3))

            gw = statep.tile([B_LOC, NJ * 12], F32)
            gw4 = gw[:, :].rearrange("p (j m n) -> p j m n", m=3, n=4)
            V.tensor_copy(gw[:, 0:12], gl[:, 0:12])
            fktmp = scrp.tile([B_LOC, 3 * 12], F32, tag="fktmp")
            for (c0, ncld, p0, bc) in FK_GROUPS:
                child = gw4[:, c0:c0 + ncld]
                loc = gl4[:, c0:c0 + ncld]
                par = gw4[:, p0:p0 + (1 if bc else ncld)]
                tmpv = fktmp[:, 0:ncld * 12].rearrange("p (j m n) -> p j m n", m=3, n=4)
                shp = [B_LOC, ncld, 3, 4]
                for k in range(3):
                    in0 = loc[:, :, k:k + 1, :].broadcast_to(shp)
                    pk = par[:, 0:1, :, k:k + 1] if bc else par[:, :, :, k:k + 1]
                    in1 = pk.broadcast_to(shp)
                    if k == 0:
                        V.tensor_mul(child[:, :, :, :], in0, in1)
                    else:
                        V.tensor_mul(tmpv, in0, in1)
                        V.tensor_add(child[:, :, :, :], child[:, :, :, :], tmpv)
                ptr = par[:, 0:1, :, 3:4] if bc else par[:, :, :, 3:4]
                V.tensor_add(child[:, :, :, 3:4], child[:, :, :, 3:4],
                             ptr.broadcast_to([B_LOC, ncld, 3, 1]))

            ct = scrp.tile([B_LOC, 72], F32, tag="ct")
            ct2 = scrp.tile([B_LOC, 72], F32, tag="ct2")
            ctv = ct[:, :].rearrange("p (j m) -> p j m", m=3).unsqueeze(3)
            ct2v = ct2[:, :].rearrange("p (j m) -> p j m", m=3).unsqueeze(3)
            for k in range(3):
                jk = jv[:, :, k:k + 1].unsqueeze(2).broadcast_to([B_LOC, NJ, 3, 1])
                if k == 0:
                    V.tensor_mul(ctv, gw4[:, :, :, k:k + 1], jk)
                else:
                    V.tensor_mul(ct2v, gw4[:, :, :, k:k + 1], jk)
                    V.tensor_add(ctv, ctv, ct2v)
            V.tensor_sub(gw4[:, :, :, 3:4], gw4[:, :, :, 3:4], ctv)

            if cfg["debug"]:
                nc.sync.dma_start(dbg["r9"].ap(), r9[:, :])
                nc.sync.dma_start(dbg["j"].ap(), j_sb[:, :])
                nc.sync.dma_start(dbg["gw"].ap(), gw[:, :])

            gat = statep.tile([NJ, 12 * B_LOC], CDT)
            gwe = gw[:, :].rearrange("p (j e) -> p e j", e=12)
            with tc.tile_pool(name="psT", bufs=3, space="PSUM") as psT:
                for e in range(12):
                    pgt = psT.tile([NJ, B_LOC], F32, tag="gt")
                    nc.tensor.transpose(pgt[:, :], gwe[:, e, :], ident[:, :])
                    V.tensor_copy(gat[:, e * B_LOC:(e + 1) * B_LOC], pgt[:, :])

            dirs_ap = dirs_d.ap().rearrange("k (c v) -> k c v", c=3)
            _main_loop_v2(nc, tc, cfg, ch, ODT, dirs_ap, coeffT_a, coeffT_b, gat,
                          wt_sb, out_d)

    nc.compile()
    _CACHE[key] = nc
    return nc


def _main_loop_v2(nc, tc, cfg, ch, ODT, dirs_ap, coeffT_a, coeffT_b, gat, wt_sb, out_d):
    V = nc.vector
    S = nc.scalar
    P = nc.gpsimd
    n_chunks = (NV + ch - 1) // ch
    with (
        tc.tile_pool(name="loop", bufs=3) as loopp,
        tc.tile_pool(name="psMM", bufs=2, space="PSUM") as psMM,
        tc.tile_pool(name="psTG", bufs=2, space="PSUM") as psTG,
    ):
        for ci in range(n_chunks):
            v0 = ci * ch
            sz = min(ch, NV - v0)
            da = loopp.tile([128, 3, ch], F16, tag="da")
            nc.sync.dma_start(da[:, :, 0:sz], dirs_ap[0:128, :, v0:v0 + sz])
            db = loopp.tile([90, 3, ch], F16, tag="db")
            nc.sync.dma_start(db[:, :, 0:sz], dirs_ap[128:KC, :, v0:v0 + sz])

            vp_sb = loopp.tile([B_LOC, 3, ch], F16, tag="vp")
            for c in range(3):
                pvc = psMM.tile([B_LOC, ch], F32, tag="mm")
                nc.tensor.matmul(pvc[:, 0:sz], coeffT_a[:, :], da[:, c, 0:sz],
                                 start=True, stop=False)
                nc.tensor.matmul(pvc[:, 0:sz], coeffT_b[:, :], db[:, c, 0:sz],
                                 start=False, stop=True)
                S.copy(vp_sb[:, c, 0:sz], pvc[:, 0:sz])

            t_sbs = []
            for n in range(3):
                ptn = psTG.tile([B_LOC, 3, ch], F32, tag="tg")
                for m in range(3):
                    e = m * 4 + n
                    nc.tensor.matmul(ptn[:, m, 0:sz],
                                     gat[:, e * B_LOC:(e + 1) * B_LOC],
                                     wt_sb[:, v0:v0 + sz], start=True, stop=True)
                t_sb = loopp.tile([B_LOC, 3, ch], F16, tag=f"tsb{n}")
                if sz == ch:
                    S.copy(t_sb[:, :, :], ptn[:, :, :])
                else:
                    for m in range(3):
                        S.copy(t_sb[:, m, 0:sz], ptn[:, m, 0:sz])
                t_sbs.append(t_sb)

            pt3 = psTG.tile([B_LOC, 3, ch], F32, tag="tg")
            for m in range(3):
                e = m * 4 + 3
                nc.tensor.matmul(pt3[:, m, 0:sz],
                                 gat[:, e * B_LOC:(e + 1) * B_LOC],
                                 wt_sb[:, v0:v0 + sz], start=True, stop=True)

            tmps = []
            for n in range(3):
                tmp = loopp.tile([B_LOC, 3, ch], F16, tag=f"tmp{n}")
                vb = vp_sb[:, n, 0:sz].unsqueeze(1).broadcast_to([B_LOC, 3, sz])
                eng = P if n == 2 else V
                eng.tensor_mul(tmp[:, :, 0:sz], t_sbs[n][:, :, 0:sz], vb)
                tmps.append(tmp)
            acc = loopp.tile([B_LOC, 3, ch], F16, tag="acc")
            V.tensor_add(acc[:, :, 0:sz], tmps[0][:, :, 0:sz], pt3[:, :, 0:sz])
            V.tensor_add(acc[:, :, 0:sz], acc[:, :, 0:sz], tmps[1][:, :, 0:sz])
            out_sb = loopp.tile([B_LOC, 3, ch], ODT, tag="outsb")
            V.tensor_add(out_sb[:, :, 0:sz], acc[:, :, 0:sz], tmps[2][:, :, 0:sz])
            nc.sync.dma_start(out_d.ap()[:, :, v0:v0 + sz], out_sb[:, :, 0:sz])


def _host_prep(inputs, cfg):
    fp16 = cfg["compute"] == "fp16"
    cdt = np.float16 if fp16 else np.float32
    shapedirs = np.asarray(inputs["shapedirs"], np.float32)
    posedirs = np.asarray(inputs["posedirs"], np.float32)
    v_template = np.asarray(inputs["v_template"], np.float32)
    Jreg = np.asarray(inputs["J_regressor"], np.float32)
    weights = np.asarray(inputs["weights"], np.float32)

    dirs = np.empty((KC, VC), np.float32)
    dirs[0:10] = shapedirs.transpose(2, 1, 0).reshape(10, VC)
    dirs[10:217] = posedirs.transpose(2, 1, 0).reshape(NP, VC)
    dirs[217] = v_template.T.reshape(VC)
    js2 = np.einsum('jv,vcs->sjc', Jreg, shapedirs).reshape(10, 72)
    jtmpl = (Jreg @ v_template).reshape(1, 72)
    return {
        "dirs": np.ascontiguousarray(dirs.astype(cdt)),
        "wt": np.ascontiguousarray(weights.T.astype(cdt)),
        "js2": np.ascontiguousarray(js2),
        "jtmpl": np.ascontiguousarray(jtmpl),
        "ident": np.eye(128, dtype=np.float32),
    }


# revision 53
# speedup vs baseline: 1.0533x; 1.0117x over previous
"""SMPL (shape blend + pose blend + LBS skinning) Bass kernel for 8 TRN2 NeuronCores.

Data-parallel over batch: B=1024 -> 128 per core. All SMPL buffers replicated.

Measured HW model this kernel is built around (NTFF traces on these cores):
  - PE runs at 1.2 GHz here (no HAM ramp observed), 1 psum column/cycle,
    out <= 512 fp32 cols per matmul (1 PSUM bank), ~300ns fixed cost per
    matmul + ~420ns LDWEIGHTS+gap, partially hidden by the queue.
  - fp8 DoubleRow streams 2 packed columns/cycle -> same out-column rate as
    fp16, but doubles K capacity per pass: used for the K=230 vp matmul
    (one mm instead of two per c-plane).
  - ACTIVATE (ScalarE) is 1x, ~(N+352)/1.2 ns; DVE fp16 tensor_tensor is 2x;
    a DVE op overlapping any GpSimd op drops to ~0.5x (shared SBUF port), so
    GpSimd is left idle on purpose.

Numerics: vp in fp8e4 DoubleRow with power-of-2 row scaling and a hi/lo
compensation split for v_template; skinning matmul T in fp16.
Measured rel err 7.7e-3 vs the 2e-2 gate.
  vp (K=220 packed into 110 rows x 2 panels):
      rows = [beta/16 x shapedirs*16 | lrot/16 x posedirs*16 |
              1 x tmpl_hi | 1/16 x tmpl_lo*16 | pad]
  (110-row DMA tiles are ~2x faster per byte than 115-row ones - the DMA
  rate is sharply sensitive to partition-row count; dirs loads are issued
  at chunk-PAIR granularity, 6KB per partition row.)

Phase structure (per core):
  prologue: ACT-table warmup at t=0; Rodrigues (V+S) -> scaled coeff ->
            fp32 transposes -> coeffT8 (fp8); J matmul; FK on V
  P1 loop (overlaps FK on V): 3 DR matmuls/chunk -> 1-bank vp psum tiles ->
            per-plane S copies into persistent vp_sb [128,3,7168] fp16. The
            12 gat transposes are emitted mid-P1 (TSPLIT=12) so the PE
            reaches them right as FK finishes; remaining P1 chunks keep the
            PE queue fed.
  P2 loop:  12 fp16 matmuls/chunk into 3-bank psum n-groups (bufs=2) +
            pt3 group; S evicts all 4 groups to fp16 (incl pt3 - frees the
            psum slot fast and keeps V in 2x mode); V: 3 broadcast muls +
            3-add chain (never in-place: dst==src DVE ops run 4x slower).
Last chunk runs at its true 234-col width. Output [14, 128, 3, 512] fp16
chunk-major; host reassembles to [1024, 6890, 3] fp32.
"""

import sys
import numpy as np
import ml_dtypes

for _p in ("/opt/trn_rl_repo",):
    if _p not in sys.path:
        sys.path.append(_p)

import concourse.bass as bass
import concourse.tile as tile
import concourse.mybir as mybir
from concourse import bacc
from concourse.bass_utils import run_bass_kernel_spmd
from concourse.alu_op_type import AluOpType

F32 = mybir.dt.float32
F16 = mybir.dt.float16
F8 = mybir.dt.float8e4
F8NP = ml_dtypes.float8_e4m3
DR = mybir.MatmulPerfMode.DoubleRow

N_CORES = 8
B = 1024
B_LOC = B // N_CORES  # 128
NV = 6890
NVP = 7168            # padded to 14*512
NCH = 14
CH = 512
NJ = 24
NP = 207

KVP = 220             # vp contraction (padded even): 10+207+2+1
KVH = KVP // 2        # 110
KT = 64               # T contraction rows (x2 panels = 128 logical);
                      # blocks at 32-aligned bases: hi @0, lo8 @32, hi16 @0/p1

# FK level groups: (child_start, n_children, parent_start, parent_broadcast)
FK_GROUPS = [
    (1, 3, 0, True),
    (4, 3, 1, False),
    (7, 3, 4, False),
    (10, 3, 7, False),
    (13, 2, 9, True),
    (15, 3, 12, False),
    (18, 2, 16, False),
    (20, 2, 18, False),
    (22, 2, 20, False),
]

CFG = {
    "compute": "fp8",    # "fp8" | "fp16" (legacy)
    "ch": 512,
    "out16": True,
    "vp_fp8": False,
    "debug": False,
    "trace": False,
}

_CACHE = {}


def _rodrigues_and_coeff(nc, tc, constp, statep, scrp, pose_sb, beta_sb):
    """Rodrigues rotation build (fp32) + scaled coeff [128, KVP] fp32.

    coeff rows: 0..9 beta/16, 10..216 lrot/16, 217 = 1.0, 218 = 1/16,
    219 = 0.
    Returns (r9, coeff, beta_sb)."""
    V = nc.vector
    S = nc.scalar

    # dummy activation at t=0: pulls the Sqrt ACT-table load off the
    # Rodrigues critical path (it overlaps the pose DMA instead). Only the
    # Sqrt set is warmed: warming Sin too would thrash (the table RAM holds
    # one set at a time, so Sqrt would reload on the path; measured 4 loads
    # instead of 2).
    warm = scrp.tile([B_LOC, 1], F32, tag="warm")
    V.memset(warm[:, :], 1.0)
    S.activation(warm[:, :], warm[:, :], mybir.ActivationFunctionType.Sqrt)

    sq = scrp.tile([B_LOC, 72], F32, tag="sq")
    V.tensor_mul(sq[:, :], pose_sb[:, :], pose_sb[:, :])
    sq3 = sq[:, :].rearrange("p (j c) -> p c j", c=3)
    th2 = scrp.tile([B_LOC, NJ], F32, tag="th2")
    V.tensor_add(th2[:, :], sq3[:, 0, :], sq3[:, 1, :])
    V.tensor_add(th2[:, :], th2[:, :], sq3[:, 2, :])
    cbias = constp.tile([128, 2], F32)
    V.memset(cbias[:, 0:1], 1e-8)
    V.memset(cbias[:, 1:2], float(np.pi / 2))
    theta = scrp.tile([B_LOC, NJ], F32, tag="theta")
    S.activation(theta[:, :], th2[:, :], mybir.ActivationFunctionType.Sqrt,
                 bias=cbias[0:B_LOC, 0:1])
    invt = scrp.tile([B_LOC, NJ], F32, tag="invt")
    V.reciprocal(invt[:, :], theta[:, :])
    sh = scrp.tile([B_LOC, NJ], F32, tag="sh")
    S.activation(sh[:, :], theta[:, :], mybir.ActivationFunctionType.Sin, scale=0.5)
    chh = scrp.tile([B_LOC, NJ], F32, tag="chh")
    S.activation(chh[:, :], theta[:, :], mybir.ActivationFunctionType.Sin,
                 scale=0.5, bias=cbias[0:B_LOC, 1:2])
    s_t = scrp.tile([B_LOC, NJ], F32, tag="s_t")
    V.scalar_tensor_tensor(s_t[:, :], sh[:, :], 2.0, chh[:, :], AluOpType.mult, AluOpType.mult)
    shsq = scrp.tile([B_LOC, NJ], F32, tag="shsq")
    V.tensor_mul(shsq[:, :], sh[:, :], sh[:, :])
    c_t = scrp.tile([B_LOC, NJ], F32, tag="c_t")
    V.tensor_scalar(c_t[:, :], shsq[:, :], -2.0, 1.0, AluOpType.mult, AluOpType.add)
    omc = scrp.tile([B_LOC, NJ], F32, tag="omc")
    V.tensor_scalar_mul(omc[:, :], shsq[:, :], 2.0)
    ax = scrp.tile([B_LOC, 72], F32, tag="ax")
    ax3 = ax[:, :].rearrange("p (j c) -> p c j", c=3)
    p3 = pose_sb[:, :].rearrange("p (j c) -> p c j", c=3)
    V.tensor_mul(ax3[:, :, :], p3[:, :, :],
                 invt[:, :].unsqueeze(1).broadcast_to([B_LOC, 3, NJ]))
    # batched outer products: [xx yy zz xy xz yz sx sy sz] in 5 ops instead
    # of 15 (each ~180ns of serial V time on the coeffT8 critical chain)
    pr9 = scrp.tile([B_LOC, 9, NJ], F32, tag="pr9")
    prm = scrp.tile([B_LOC, 6, NJ], F32, tag="prm")
    V.tensor_mul(pr9[:, 0:3, :], ax3[:, 0:3, :], ax3[:, 0:3, :])
    V.tensor_mul(pr9[:, 3:5, :],
                 ax3[:, 0:1, :].broadcast_to([B_LOC, 2, NJ]), ax3[:, 1:3, :])
    V.tensor_mul(pr9[:, 5:6, :], ax3[:, 1:2, :], ax3[:, 2:3, :])
    V.tensor_mul(prm[:, :, :], pr9[:, 0:6, :],
                 omc[:, :].unsqueeze(1).broadcast_to([B_LOC, 6, NJ]))
    V.tensor_mul(pr9[:, 6:9, :],
                 s_t[:, :].unsqueeze(1).broadcast_to([B_LOC, 3, NJ]),
                 ax3[:, 0:3, :])
    prods = {"xx": prm[:, 0, :], "yy": prm[:, 1, :], "zz": prm[:, 2, :],
             "xy": prm[:, 3, :], "xz": prm[:, 4, :], "yz": prm[:, 5, :],
             "sx": pr9[:, 6, :], "sy": pr9[:, 7, :], "sz": pr9[:, 8, :]}
    r9 = statep.tile([B_LOC, NJ * 9], F32)
    r9e = r9[:, :].rearrange("p (j e) -> p e j", e=9)
    ENTRIES = [
        ("add", "c", "xx"), ("sub", "xy", "sz"), ("add", "xz", "sy"),
        ("add", "xy", "sz"), ("add", "c", "yy"), ("sub", "yz", "sx"),
        ("sub", "xz", "sy"), ("add", "yz", "sx"), ("add", "c", "zz"),
    ]
    for e, (op, a, b_) in enumerate(ENTRIES):
        ta = c_t[:, :] if a == "c" else prods[a]
        fn = V.tensor_add if op == "add" else V.tensor_sub
        fn(r9e[:, e, :], ta, prods[b_])

    # ---- scaled coeff ----
    coeff = statep.tile([B_LOC, KVP], F32)
    V.tensor_scalar_mul(coeff[:, 0:10], beta_sb[:, :], 1.0 / 16.0)
    # lrot/16 with diag -1/16
    V.tensor_scalar_mul(coeff[:, 10:217], r9[:, 9:216], 1.0 / 16.0)
    lr9 = coeff[:, 10:217].rearrange("p (j e) -> p e j", e=9)
    for e in (0, 4, 8):
        V.tensor_scalar_add(lr9[:, e, :], lr9[:, e, :], -1.0 / 16.0)
    V.memset(coeff[:, 217:218], 1.0)
    V.memset(coeff[:, 218:219], 1.0 / 16.0)
    V.memset(coeff[:, 219:220], 0.0)
    return r9, coeff, beta_sb


def build_program_fp8(cfg):
    key = ("fp8", cfg["debug"])
    if key in _CACHE:
        return _CACHE[key]

    nc = bacc.Bacc("TRN2", target_bir_lowering=False, debug=False)

    pose_d = nc.dram_tensor("pose", [B_LOC, 72], F32, kind="ExternalInput")
    beta_d = nc.dram_tensor("beta", [B_LOC, 10], F32, kind="ExternalInput")
    # chunk-PAIR granularity: 6144B per partition row per DMA (115 x 3KB
    # descriptors measured only ~23GB/s/engine; P1 was DMA-bandwidth bound
    # at 2.85us/chunk)
    dirs8_d = nc.dram_tensor("dirs8", [NCH // 2, KVH, 2, 2, 3, CH], F8,
                             kind="ExternalInput")
    # wt replicated at partition bases 0/32/64: T matmuls rotate across
    # three 32-row PE quadrants so LDWEIGHTS can be pulled ahead into idle
    # rows while the previous matmul streams (re-validated: the earlier
    # "regression" was device throttling; fast-equivalent T-mm is ~600 vs
    # 625ns)
    wt16_d = nc.dram_tensor("wt16", [NCH, 128, CH], F16, kind="ExternalInput")
    js2_d = nc.dram_tensor("js2", [10, 72], F32, kind="ExternalInput")
    jtmpl_d = nc.dram_tensor("jtmpl", [1, 72], F32, kind="ExternalInput")
    ident_d = nc.dram_tensor("ident", [128, 128], F32, kind="ExternalInput")
    out_d = nc.dram_tensor("out", [NCH, B_LOC, 3, CH], F16, kind="ExternalOutput")
    dbg = {}
    if cfg["debug"]:
        dbg["r9"] = nc.dram_tensor("dbg_r9", [B_LOC, 216], F32, kind="ExternalOutput")
        dbg["j"] = nc.dram_tensor("dbg_j", [B_LOC, 72], F32, kind="ExternalOutput")
        dbg["gw"] = nc.dram_tensor("dbg_gw", [B_LOC, 288], F32, kind="ExternalOutput")
        dbg["vp"] = nc.dram_tensor("dbg_vp", [B_LOC, 3, NVP], F16, kind="ExternalOutput")

    with tile.TileContext(nc) as tc:
        with (
            tc.tile_pool(name="const", bufs=1) as constp,
            tc.tile_pool(name="state", bufs=1) as statep,
            tc.tile_pool(name="scr", bufs=1) as scrp,
        ):
            V = nc.vector
            S = nc.scalar
            # pose/beta first on the sync queue: they gate the whole
            # Rodrigues critical chain (ident/js2/jtmpl aren't needed until
            # the transposes ~10us later)
            pose_sb = statep.tile([B_LOC, 72], F32)
            nc.sync.dma_start(pose_sb[:, :], pose_d.ap())
            beta_sb = statep.tile([B_LOC, 10], F32)
            nc.sync.dma_start(beta_sb[:, :], beta_d.ap())
            ident = constp.tile([128, 128], F32)
            nc.sync.dma_start(ident[:, :], ident_d.ap())
            js2 = statep.tile([10, 72], F32)
            nc.sync.dma_start(js2[:, :], js2_d.ap())
            jtmpl = statep.tile([1, 72], F32)
            nc.sync.dma_start(jtmpl[:, :], jtmpl_d.ap())

            r9, coeff, beta_sb = _rodrigues_and_coeff(
                nc, tc, constp, statep, scrp, pose_sb, beta_sb)

            # ---- coeffT8 [115, 2, 128] via two fp32 transposes + fp8 cast ----
            coeffT8 = statep.tile([KVH, 2, B_LOC], F8)
            betaT = statep.tile([10, B_LOC], F32)
            j_sb = statep.tile([B_LOC, 72], F32)
            with tc.tile_pool(name="psA", bufs=2, space="PSUM") as psA:
                ptA = psA.tile([KVH, 128], F32, tag="tp")
                nc.tensor.transpose(ptA[:, :], coeff[:, 0:KVH], ident[:, :])
                V.tensor_copy(coeffT8[:, 0, :], ptA[:, 0:B_LOC])
                V.tensor_scalar_mul(betaT[:, :], ptA[0:10, 0:B_LOC], 16.0)
                ptB = psA.tile([KVH, 128], F32, tag="tp")
                nc.tensor.transpose(ptB[:, :], coeff[:, KVH:KVP], ident[:, :])
                V.tensor_copy(coeffT8[:, 1, :], ptB[:, 0:B_LOC])

                # ---- J = [beta | 1] @ [JS2 ; Jtmpl] ----
                pj = psA.tile([B_LOC, 72], F32, tag="pj")
                onesT = statep.tile([1, B_LOC], F32)
                V.memset(onesT[0:1, :], 1.0)
                nc.tensor.matmul(pj[:, :], betaT[:, :], js2[:, :], start=True, stop=False)
                nc.tensor.matmul(pj[:, :], onesT[0:1, :], jtmpl[0:1, :], start=False, stop=True)
                V.tensor_copy(j_sb[:, :], pj[:, :])

            # ---- P1: vp matmuls (independent of FK; emitted before gat
            # transposes so the PE queue is not blocked behind FK).
            # _p1_chunk is invoked for chunks 0..TSPLIT-1 here and the rest
            # after the gat transposes, so the transposes (which wait on FK)
            # slot into the PE queue right when FK finishes. ----
            vp_sb = statep.tile([B_LOC, 3, NVP], F16)
            p1_ctx = ctx = tc.tile_pool(name="p1", bufs=4)
            p1p = ctx.__enter__()
            # wide 3-bank vp psum tiles with ONE wide S copy per chunk:
            # ScalarE is the global wall (86% busy), so minimizing ACT
            # per-op overhead beats finer psum-release granularity
            psVP_ctx = tc.tile_pool(name="psVP", bufs=2, space="PSUM")
            psVP = psVP_ctx.__enter__()

            da8_pair = [None]

            def _p1_chunk(ci):
                sz = min(CH, NV - ci * CH)
                if ci % 2 == 0:
                    da8 = p1p.tile([KVH, 2, 2, 3, CH], F8, tag="da")
                    da8_pair[0] = da8
                    nc.sync.dma_start(da8[:, :, :, :, :],
                                      dirs8_d.ap()[ci // 2])
                da8 = da8_pair[0]
                pvc = psVP.tile([B_LOC, 3, CH], F32, tag="vp")
                for c in range(3):
                    nc.tensor.matmul(pvc[:, c, 0:sz], coeffT8[:, :, :],
                                     da8[:, ci % 2, :, c, 0:sz], start=True,
                                     stop=True, perf_mode=DR)
                if ci >= 11:
                    # FK is done by now: V is idle in late P1, S is the
                    # global wall -- V takes the last three vp evictions
                    V.tensor_copy(vp_sb[:, :, ci * CH:ci * CH + sz],
                                  pvc[:, :, 0:sz])
                else:
                    S.copy(vp_sb[:, :, ci * CH:ci * CH + sz], pvc[:, :, 0:sz])

            TSPLIT = 12
            for ci in range(TSPLIT):
                _p1_chunk(ci)

            # ---- J_rel ----
            jrel = statep.tile([B_LOC, 72], F32)
            jv = j_sb[:, :].rearrange("p (j c) -> p j c", c=3)
            jrv = jrel[:, :].rearrange("p (j c) -> p j c", c=3)
            V.tensor_copy(jrel[:, 0:3], j_sb[:, 0:3])
            V.tensor_sub(jrv[:, 1:4], jv[:, 1:4], jv[:, 0:1].broadcast_to([B_LOC, 3, 3]))
            V.tensor_sub(jrv[:, 4:12], jv[:, 4:12], jv[:, 1:9])
            V.tensor_sub(jrv[:, 12:15], jv[:, 12:15], jv[:, 9:10].broadcast_to([B_LOC, 3, 3]))
            V.tensor_sub(jrv[:, 15:18], jv[:, 15:18], jv[:, 12:15])
            V.tensor_sub(jrv[:, 18:24], jv[:, 18:24], jv[:, 16:22])

            # ---- local transforms Gl [128, 24*12] (3x4 row-major [R|t]) ----
            gl = statep.tile([B_LOC, NJ * 12], F32)
            gl4 = gl[:, :].rearrange("p (j m n) -> p j m n", m=3, n=4)
            r94 = r9[:, :].rearrange("p (j m n) -> p j m n", m=3, n=3)
            V.tensor_copy(gl4[:, :, :, 0:3], r94[:, :, :, :])
            V.tensor_copy(gl4[:, :, :, 3:4], jrv[:, :, :].unsqueeze(3))

            # ---- forward kinematics ----
            gw = statep.tile([B_LOC, NJ * 12], F32)
            gw4 = gw[:, :].rearrange("p (j m n) -> p j m n", m=3, n=4)
            V.tensor_copy(gw[:, 0:12], gl[:, 0:12])
            fktmp = scrp.tile([B_LOC, 3 * 12], F32, tag="fktmp")
            for (c0, ncld, p0, bc) in FK_GROUPS:
                child = gw4[:, c0:c0 + ncld]
                loc = gl4[:, c0:c0 + ncld]
                par = gw4[:, p0:p0 + (1 if bc else ncld)]
                tmpv = fktmp[:, 0:ncld * 12].rearrange("p (j m n) -> p j m n", m=3, n=4)
                shp = [B_LOC, ncld, 3, 4]
                for k in range(3):
                    in0 = loc[:, :, k:k + 1, :].broadcast_to(shp)
                    pk = par[:, 0:1, :, k:k + 1] if bc else par[:, :, :, k:k + 1]
                    in1 = pk.broadcast_to(shp)
                    if k == 0:
                        V.tensor_mul(child[:, :, :, :], in0, in1)
                    else:
                        V.tensor_mul(tmpv, in0, in1)
                        V.tensor_add(child[:, :, :, :], child[:, :, :, :], tmpv)
                ptr = par[:, 0:1, :, 3:4] if bc else par[:, :, :, 3:4]
                V.tensor_add(child[:, :, :, 3:4], child[:, :, :, 3:4],
                             ptr.broadcast_to([B_LOC, ncld, 3, 1]))

            # ---- rest-pose correction: t_j -= R_j^w @ J_j ----
            ct = scrp.tile([B_LOC, 72], F32, tag="ct")
            ct2 = scrp.tile([B_LOC, 72], F32, tag="ct2")
            ctv = ct[:, :].rearrange("p (j m) -> p j m", m=3).unsqueeze(3)
            ct2v = ct2[:, :].rearrange("p (j m) -> p j m", m=3).unsqueeze(3)
            for k in range(3):
                jk = jv[:, :, k:k + 1].unsqueeze(2).broadcast_to([B_LOC, NJ, 3, 1])
                if k == 0:
                    V.tensor_mul(ctv, gw4[:, :, :, k:k + 1], jk)
                else:
                    V.tensor_mul(ct2v, gw4[:, :, :, k:k + 1], jk)
                    V.tensor_add(ctv, ctv, ct2v)
            V.tensor_sub(gw4[:, :, :, 3:4], gw4[:, :, :, 3:4], ctv)

            if cfg["debug"]:
                nc.sync.dma_start(dbg["r9"].ap(), r9[:, :])
                nc.sync.dma_start(dbg["j"].ap(), j_sb[:, :])
                nc.sync.dma_start(dbg["gw"].ap(), gw[:, :])

            # ---- gat16 via 12 fp32 transposes: [24, 12, 128] fp16 ----
            gat16 = statep.tile([NJ, 12, B_LOC], F16)
            gwe = gw[:, :].rearrange("p (j e) -> p e j", e=12)
            with tc.tile_pool(name="psT", bufs=2, space="PSUM") as psT:
                for e in range(12):
                    pgt = psT.tile([NJ, B_LOC], F32, tag="gt")
                    nc.tensor.transpose(pgt[:, :], gwe[:, e, :], ident[:, :])
                    V.tensor_copy(gat16[:, e, :], pgt[:, :])
            # replicate gat at the three quadrant bases (base 96 is rejected
            # by bass) for the quadrant-rotated T matmuls
            gat16x = statep.tile([96, 12, B_LOC], F16)
            for q in range(3):
                nc.sync.dma_start(gat16x[32 * q:32 * q + NJ, :, :],
                                  gat16[:, :, :])
            # remaining P1 chunks fill the PE queue behind the transposes
            for ci in range(TSPLIT, NCH):
                _p1_chunk(ci)
            psVP_ctx.__exit__(None, None, None)
            p1_ctx.__exit__(None, None, None)

            if cfg["debug"]:
                nc.sync.dma_start(dbg["vp"].ap(), vp_sb[:, :, :])

            # ---- P2: skinning matmuls + combine ----
            with (
                tc.tile_pool(name="p2", bufs=3) as p2p,
                tc.tile_pool(name="psTG", bufs=2, space="PSUM") as psTG,
            ):
                qi = 0
                for ci in range(NCH):
                    v0 = ci * CH
                    sz = min(CH, NV - v0)
                    wt16c = p2p.tile([128, CH], F16, tag="wt")
                    nc.sync.dma_start(wt16c[:, :], wt16_d.ap()[ci])

                    t_sbs = []
                    for n in range(3):
                        ptn = psTG.tile([B_LOC, 3, CH], F32, tag="tg")
                        for m in range(3):
                            e = m * 4 + n
                            q = qi % 3
                            qi += 1
                            nc.tensor.matmul(
                                ptn[:, m, 0:sz],
                                gat16x[32 * q:32 * q + NJ, e, :],
                                wt16c[32 * q:32 * q + NJ, 0:sz],
                                start=True, stop=True)
                        t_sb = p2p.tile([B_LOC, 3, CH], F16, tag=f"tsb{n}")
                        S.copy(t_sb[:, :, 0:sz], ptn[:, :, 0:sz])
                        t_sbs.append(t_sb)

                    pt3 = psTG.tile([B_LOC, 3, CH], F32, tag="tg")
                    for m in range(3):
                        e = m * 4 + 3
                        q = qi % 3
                        qi += 1
                        nc.tensor.matmul(pt3[:, m, 0:sz],
                                         gat16x[32 * q:32 * q + NJ, e, :],
                                         wt16c[32 * q:32 * q + NJ, 0:sz],
                                         start=True, stop=True)
                    # pt3 evicted by S too: frees its psum slot fast (PE would
                    # otherwise stall on the rotation) and keeps the V add in
                    # fp16 2x mode. GpSimd is NOT used: it shares the DVE SBUF
                    # port, halving any concurrent 2-port V op (measured
                    # 950ns -> 3200ns).
                    pt3_sb = p2p.tile([B_LOC, 3, CH], F16, tag="pt3sb")
                    if ci in (4, 9):
                        # level S (99.5% busy) vs V (90.5%): V takes 2 of the
                        # 14 pt3 evictions
                        V.tensor_copy(pt3_sb[:, :, 0:sz], pt3[:, :, 0:sz])
                    else:
                        S.copy(pt3_sb[:, :, 0:sz], pt3[:, :, 0:sz])

                    tmps = []
                    for n in range(3):
                        tmp = p2p.tile([B_LOC, 3, CH], F16, tag=f"tmp{n}")
                        vb = vp_sb[:, n, v0:v0 + sz].unsqueeze(1).broadcast_to(
                            [B_LOC, 3, sz])
                        V.tensor_mul(tmp[:, :, 0:sz], t_sbs[n][:, :, 0:sz], vb)
                        tmps.append(tmp)
                    acc = p2p.tile([B_LOC, 3, CH], F16, tag="acc")
                    V.tensor_add(acc[:, :, 0:sz], tmps[0][:, :, 0:sz],
                                 pt3_sb[:, :, 0:sz])
                    acc2 = p2p.tile([B_LOC, 3, CH], F16, tag="acc2")
                    V.tensor_add(acc2[:, :, 0:sz], acc[:, :, 0:sz],
                                 tmps[1][:, :, 0:sz])
                    out_sb = p2p.tile([B_LOC, 3, CH], F16, tag="outsb")
                    V.tensor_add(out_sb[:, :, 0:sz], acc2[:, :, 0:sz],
                                 tmps[2][:, :, 0:sz])
                    nc.sync.dma_start(out_d.ap()[ci][:, :, 0:sz],
                                      out_sb[:, :, 0:sz])

    nc.compile()
    _CACHE[key] = nc
    return nc


def _host_prep_fp8(inputs):
    f32 = np.float32
    shapedirs = np.asarray(inputs["shapedirs"], f32)   # [V,3,10]
    posedirs = np.asarray(inputs["posedirs"], f32)     # [V,3,207]
    v_template = np.asarray(inputs["v_template"], f32)  # [V,3]
    Jreg = np.asarray(inputs["J_regressor"], f32)       # [24,V]
    weights = np.asarray(inputs["weights"], f32)        # [V,24]

    dirs = np.zeros((KVP, 3, NVP), f32)
    sd = shapedirs.transpose(2, 1, 0)   # [10,3,V]
    pd = posedirs.transpose(2, 1, 0)    # [207,3,V]
    dirs[0:10, :, :NV] = sd * 16.0
    dirs[10:217, :, :NV] = pd * 16.0
    tmpl = v_template.T
    hi8 = tmpl.astype(F8NP).astype(f32)
    dirs[217, :, :NV] = hi8
    dirs[218, :, :NV] = (tmpl - hi8) * 16.0
    dirs8 = dirs.astype(F8NP)
    d = dirs8.reshape(KVP, 3, NCH, CH)
    dirs8_arr = np.empty((NCH, KVH, 2, 3, CH), F8NP)
    dirs8_arr[:, :, 0] = d[0:KVH].transpose(2, 0, 1, 3)
    dirs8_arr[:, :, 1] = d[KVH:KVP].transpose(2, 0, 1, 3)
    # [NCH,...] -> chunk-pair-major [NCH/2, KVH, 2(chunk), 2(panel), 3, CH]
    dirs8_arr = dirs8_arr.reshape(NCH // 2, 2, KVH, 2, 3, CH).transpose(
        0, 2, 1, 3, 4, 5)

    wt = np.zeros((NJ, NVP), np.float16)
    wt[:, :NV] = weights.T.astype(np.float16)
    wtc = wt.reshape(NJ, NCH, CH).transpose(1, 0, 2)    # [NCH, NJ, CH]
    wt16_arr = np.zeros((NCH, 128, CH), np.float16)
    for q in range(3):
        wt16_arr[:, 32 * q:32 * q + NJ] = wtc
    wt16_arr = np.ascontiguousarray(wt16_arr)

    js2 = np.einsum('jv,vcs->sjc', Jreg, shapedirs).reshape(10, 72)
    jtmpl = (Jreg @ v_template).reshape(1, 72)
    return {
        "dirs8": np.ascontiguousarray(dirs8_arr),
        "wt16": wt16_arr,
        "js2": np.ascontiguousarray(js2),
        "jtmpl": np.ascontiguousarray(jtmpl),
        "ident": np.eye(128, dtype=f32),
    }


def kernel(pose, beta, shapedirs, posedirs, v_template, J_regressor, weights):
    cfg = CFG
    if cfg["compute"] == "fp8":
        nc = build_program_fp8(cfg)
        rep = _host_prep_fp8(dict(shapedirs=shapedirs, posedirs=posedirs,
                                  v_template=v_template, J_regressor=J_regressor,
                                  weights=weights))
    else:
        nc = build_program(cfg)
        rep = _host_prep(dict(shapedirs=shapedirs, posedirs=posedirs,
                              v_template=v_template, J_regressor=J_regressor,
                              weights=weights), cfg)
    pose = np.asarray(pose, np.float32)
    beta = np.asarray(beta, np.float32)
    in_maps = []
    for i in range(N_CORES):
        m = dict(rep)
        m["pose"] = np.ascontiguousarray(pose[i * B_LOC:(i + 1) * B_LOC])
        m["beta"] = np.ascontiguousarray(beta[i * B_LOC:(i + 1) * B_LOC])
        in_maps.append(m)
    res = run_bass_kernel_spmd(nc, in_maps, core_ids=list(range(N_CORES)),
                               trace=cfg.get("trace", False))
    kernel.last_results = res
    outs = []
    for i in range(N_CORES):
        o = np.asarray(res.results[i]["out"], np.float32)
        if cfg["compute"] == "fp8":
            # [NCH, 128, 3, CH] -> [128, 3, NVP] -> [128, NV, 3]
            o = o.transpose(1, 2, 0, 3).reshape(B_LOC, 3, NVP)[:, :, :NV]
        outs.append(o.transpose(0, 2, 1))
    return np.ascontiguousarray(np.concatenate(outs, axis=0))


# ---------------------------------------------------------------------------
# Legacy fp16 path (kept for A/B testing via CFG["compute"]="fp16")
# ---------------------------------------------------------------------------
KC = 218
VC = 3 * NV


def build_program(cfg):
    key = (cfg["compute"], cfg["ch"], cfg["out16"], cfg["debug"])
    if key in _CACHE:
        return _CACHE[key]

    fp16 = cfg["compute"] == "fp16"
    CDT = F16 if fp16 else F32
    ODT = F16 if (fp16 and cfg["out16"]) else F32
    ch = cfg["ch"] if fp16 else 256

    nc = bacc.Bacc("TRN2", target_bir_lowering=False, debug=False)

    pose_d = nc.dram_tensor("pose", [B_LOC, 72], F32, kind="ExternalInput")
    beta_d = nc.dram_tensor("beta", [B_LOC, 10], F32, kind="ExternalInput")
    dirs_d = nc.dram_tensor("dirs", [KC, VC], CDT, kind="ExternalInput")
    wt_d = nc.dram_tensor("wt", [NJ, NV], CDT, kind="ExternalInput")
    js2_d = nc.dram_tensor("js2", [10, 72], F32, kind="ExternalInput")
    jtmpl_d = nc.dram_tensor("jtmpl", [1, 72], F32, kind="ExternalInput")
    ident_d = nc.dram_tensor("ident", [128, 128], F32, kind="ExternalInput")
    out_d = nc.dram_tensor("out", [B_LOC, 3, NV], ODT, kind="ExternalOutput")
    dbg = {}
    if cfg["debug"]:
        dbg["r9"] = nc.dram_tensor("dbg_r9", [B_LOC, 216], F32, kind="ExternalOutput")
        dbg["j"] = nc.dram_tensor("dbg_j", [B_LOC, 72], F32, kind="ExternalOutput")
        dbg["gw"] = nc.dram_tensor("dbg_gw", [B_LOC, 288], F32, kind="ExternalOutput")
        dbg["vp"] = nc.dram_tensor("dbg_vp", [B_LOC, 3, NV], F32, kind="ExternalOutput")

    with tile.TileContext(nc) as tc:
        with (
            tc.tile_pool(name="const", bufs=1) as constp,
            tc.tile_pool(name="state", bufs=1) as statep,
            tc.tile_pool(name="scr", bufs=1) as scrp,
        ):
            ident = constp.tile([128, 128], F32)
            nc.sync.dma_start(ident[:, :], ident_d.ap())
            wt_sb = constp.tile([NJ, NV], CDT)
            nc.sync.dma_start(wt_sb[:, :], wt_d.ap())
            js2 = statep.tile([10, 72], F32)
            nc.sync.dma_start(js2[:, :], js2_d.ap())
            jtmpl = statep.tile([1, 72], F32)
            nc.sync.dma_start(jtmpl[:, :], jtmpl_d.ap())
            pose_sb = statep.tile([B_LOC, 72], F32)
            nc.sync.dma_start(pose_sb[:, :], pose_d.ap())

            V = nc.vector
            S = nc.scalar
            sq = scrp.tile([B_LOC, 72], F32, tag="sq")
            V.tensor_mul(sq[:, :], pose_sb[:, :], pose_sb[:, :])
            sq3 = sq[:, :].rearrange("p (j c) -> p c j", c=3)
            th2 = scrp.tile([B_LOC, NJ], F32, tag="th2")
            V.tensor_add(th2[:, :], sq3[:, 0, :], sq3[:, 1, :])
            V.tensor_add(th2[:, :], th2[:, :], sq3[:, 2, :])
            cbias = constp.tile([128, 2], F32)
            V.memset(cbias[:, 0:1], 1e-8)
            V.memset(cbias[:, 1:2], float(np.pi / 2))
            theta = scrp.tile([B_LOC, NJ], F32, tag="theta")
            S.activation(theta[:, :], th2[:, :], mybir.ActivationFunctionType.Sqrt,
                         bias=cbias[0:B_LOC, 0:1])
            invt = scrp.tile([B_LOC, NJ], F32, tag="invt")
            V.reciprocal(invt[:, :], theta[:, :])
            sh = scrp.tile([B_LOC, NJ], F32, tag="sh")
            S.activation(sh[:, :], theta[:, :], mybir.ActivationFunctionType.Sin, scale=0.5)
            chh = scrp.tile([B_LOC, NJ], F32, tag="chh")
            S.activation(chh[:, :], theta[:, :], mybir.ActivationFunctionType.Sin,
                         scale=0.5, bias=cbias[0:B_LOC, 1:2])
            s_t = scrp.tile([B_LOC, NJ], F32, tag="s_t")
            V.scalar_tensor_tensor(s_t[:, :], sh[:, :], 2.0, chh[:, :], AluOpType.mult, AluOpType.mult)
            shsq = scrp.tile([B_LOC, NJ], F32, tag="shsq")
            V.tensor_mul(shsq[:, :], sh[:, :], sh[:, :])
            c_t = scrp.tile([B_LOC, NJ], F32, tag="c_t")
            V.tensor_scalar(c_t[:, :], shsq[:, :], -2.0, 1.0, AluOpType.mult, AluOpType.add)
            omc = scrp.tile([B_LOC, NJ], F32, tag="omc")
            V.tensor_scalar_mul(omc[:, :], shsq[:, :], 2.0)
            ax = scrp.tile([B_LOC, 72], F32, tag="ax")
            ax3 = ax[:, :].rearrange("p (j c) -> p c j", c=3)
            p3 = pose_sb[:, :].rearrange("p (j c) -> p c j", c=3)
            for ci in range(3):
                V.tensor_mul(ax3[:, ci, :], p3[:, ci, :], invt[:, :])
            prods = {}
            for name, (a, b_) in {
                "xx": (0, 0), "yy": (1, 1), "zz": (2, 2),
                "xy": (0, 1), "xz": (0, 2), "yz": (1, 2),
            }.items():
                t = scrp.tile([B_LOC, NJ], F32, tag="prod_" + name)
                V.tensor_mul(t[:, :], ax3[:, a, :], ax3[:, b_, :])
                V.tensor_mul(t[:, :], t[:, :], omc[:, :])
                prods[name] = t
            for name, a in {"sx": 0, "sy": 1, "sz": 2}.items():
                t = scrp.tile([B_LOC, NJ], F32, tag="prod_" + name)
                V.tensor_mul(t[:, :], s_t[:, :], ax3[:, a, :])
                prods[name] = t
            r9 = statep.tile([B_LOC, NJ * 9], F32)
            r9e = r9[:, :].rearrange("p (j e) -> p e j", e=9)
            ENTRIES = [
                ("add", "c", "xx"), ("sub", "xy", "sz"), ("add", "xz", "sy"),
                ("add", "xy", "sz"), ("add", "c", "yy"), ("sub", "yz", "sx"),
                ("sub", "xz", "sy"), ("add", "yz", "sx"), ("add", "c", "zz"),
            ]
            for e, (op, a, b_) in enumerate(ENTRIES):
                ta = c_t if a == "c" else prods[a]
                fn = V.tensor_add if op == "add" else V.tensor_sub
                fn(r9e[:, e, :], ta[:, :], prods[b_][:, :])

            coeff = statep.tile([B_LOC, KC], F32)
            nc.sync.dma_start(coeff[:, 0:10], beta_d.ap())
            V.tensor_copy(coeff[:, 10:217], r9[:, 9:216])
            lr9 = coeff[:, 10:217].rearrange("p (j e) -> p e j", e=9)
            for e in (0, 4, 8):
                V.tensor_scalar_add(lr9[:, e, :], lr9[:, e, :], -1.0)
            V.memset(coeff[:, 217:218], 1.0)

            with tc.tile_pool(name="psA", bufs=2, space="PSUM") as psA:
                pt1 = psA.tile([128, 128], F32, tag="tp")
                nc.tensor.transpose(pt1[:, :], coeff[:, 0:128], ident[:, :])
                coeffT_a = statep.tile([128, B_LOC], CDT)
                V.tensor_copy(coeffT_a[:, :], pt1[:, :])
                pt2 = psA.tile([128, 128], F32, tag="tp")
                nc.tensor.transpose(pt2[0:90, :], coeff[:, 128:218], ident[:, :])
                coeffT_b = statep.tile([90, B_LOC], CDT)
                V.tensor_copy(coeffT_b[:, :], pt2[0:90, :])

                pj = psA.tile([B_LOC, 72], F32, tag="pj")
                onesT = statep.tile([1, B_LOC], F32)
                V.memset(onesT[0:1, :], 1.0)
                if fp16:
                    betaT = statep.tile([10, B_LOC], F32)
                    V.tensor_copy(betaT[:, :], pt1[0:10, :])
                    betaT_ap = betaT[:, :]
                else:
                    betaT_ap = coeffT_a[0:10, :]
                nc.tensor.matmul(pj[:, :], betaT_ap, js2[:, :], start=True, stop=False)
                nc.tensor.matmul(pj[:, :], onesT[0:1, :], jtmpl[0:1, :], start=False, stop=True)
                j_sb = statep.tile([B_LOC, 72], F32)
                V.tensor_copy(j_sb[:, :], pj[:, :])

            jrel = statep.tile([B_LOC, 72], F32)
            jv = j_sb[:, :].rearrange("p (j c) -> p j c", c=3)
            jrv = jrel[:, :].rearrange("p (j c) -> p j c", c=3)
            V.tensor_copy(jrel[:, 0:3], j_sb[:, 0:3])
            V.tensor_sub(jrv[:, 1:4], jv[:, 1:4], jv[:, 0:1].broadcast_to([B_LOC, 3, 3]))
            V.tensor_sub(jrv[:, 4:12], jv[:, 4:12], jv[:, 1:9])
            V.tensor_sub(jrv[:, 12:15], jv[:, 12:15], jv[:, 9:10].broadcast_to([B_LOC, 3, 3]))
            V.tensor_sub(jrv[:, 15:18], jv[:, 15:18], jv[:, 12:15])
            V.tensor_sub(jrv[:, 18:24], jv[:, 18:24], jv[:, 16:22])

            gl = statep.tile([B_LOC, NJ * 12], F32)
            gl4 = gl[:, :].rearrange("p (j m n) -> p j m n", m=3, n=4)
            r94 = r9[:, :].rearrange("p (j m n) -> p j m n", m=3, n=3)
            V.tensor_copy(gl4[:, :, :, 0:3], r94[:, :, :, :])
            V.tensor_copy(gl4[:, :, :, 3:4], jrv[:, :, :].unsqueeze(3))

            gw = statep.tile([B_LOC, NJ * 12], F32)
            gw4 = gw[:, :].rearrange("p (j m n) -> p j m n", m=3, n=4)
            V.tensor_copy(gw[:, 0:12], gl[:, 0:12])
            fktmp = scrp.tile([B_LOC, 3 * 12], F32, tag="fktmp")
            for (c0, ncld, p0, bc) in FK_GROUPS:
                child = gw4[:, c0:c0 + ncld]
                loc = gl4[:, c0:c0 + ncld]
                par = gw4[:, p0:p0 + (1 if bc else ncld)]
                tmpv = fktmp[:, 0:ncld * 12].rearrange("p (j m n) -> p j m n", m=3, n=4)
                shp = [B_LOC, ncld, 3, 4]
                for k in range(3):
                    in0 = loc[:, :, k:k + 1, :].broadcast_to(shp)
                    pk = par[:, 0:1, :, k:k + 1] if bc else par[:, :, :, k:k + 1]
                    in1 = pk.broadcast_to(shp)
                    if k == 0:
                        V.tensor_mul(child[:, :, :, :], in0, in1)
                    else:
                        V.tensor_mul(tmpv, in0, in1)
                        V.tensor_add(child[:, :, :, :], child[:, :, :, :], tmpv)
                ptr = par[:, 0:1, :, 3:4] if bc else par[:, :, :, 3:4]
                V.tensor_add(child[:, :, :, 3:4], child[:, :, :, 3:4],
                             ptr.broadcast_to([B_LOC, ncld, 3, 1]))

            ct = scrp.tile([B_LOC, 72], F32, tag="ct")
            ct2 = scrp.tile([B_LOC, 72], F32, tag="ct2")
            ctv = ct[:, :].rearrange("p (j m) -> p j m", m=3).unsqueeze(3)
            ct2v = ct2[:, :].rearrange("p (j m) -> p j m", m=3).unsqueeze(3)
            for k in range(3):
                jk = jv[:, :, k:k + 1].unsqueeze(2).broadcast_to([B_LOC, NJ, 3, 1])
                if k == 0:
                    V.tensor_mul(ctv, gw4[:, :, :, k:k + 1], jk)
                else:
                    V.tensor_mul(ct2v, gw4[:, :, :, k:k + 1], jk)
                    V.tensor_add(ctv, ctv, ct2v)
            V.tensor_sub(gw4[:, :, :, 3:4], gw4[:, :, :, 3:4], ctv)

            if cfg["debug"]:
                nc.sync.dma_start(dbg["r9"].ap(), r9[:, :])
                nc.sync.dma_start(dbg["j"].ap(), j_sb[:, :])
                nc.sync.dma_start(dbg["gw"].ap(), gw[:, :])

            gat = statep.tile([NJ, 12 * B_LOC], CDT)
            gwe = gw[:, :].rearrange("p (j e) -> p e j", e=12)
            with tc.tile_pool(name="psT", bufs=3, space="PSUM") as psT:
                for e in range(12):
                    pgt = psT.tile([NJ, B_LOC], F32, tag="gt")
                    nc.tensor.transpose(pgt[:, :], gwe[:, e, :], ident[:, :])
                    V.tensor_copy(gat[:, e * B_LOC:(e + 1) * B_LOC], pgt[:, :])

            dirs_ap = dirs_d.ap().rearrange("k (c v) -> k c v", c=3)
            _main_loop_v2(nc, tc, cfg, ch, ODT, dirs_ap, coeffT_a, coeffT_b, gat,
                          wt_sb, out_d)

    nc.compile()
    _CACHE[key] = nc
    return nc


def _main_loop_v2(nc, tc, cfg, ch, ODT, dirs_ap, coeffT_a, coeffT_b, gat, wt_sb, out_d):
    V = nc.vector
    S = nc.scalar
    P = nc.gpsimd
    n_chunks = (NV + ch - 1) // ch
    with (
        tc.tile_pool(name="loop", bufs=3) as loopp,
        tc.tile_pool(name="psMM", bufs=2, space="PSUM") as psMM,
        tc.tile_pool(name="psTG", bufs=2, space="PSUM") as psTG,
    ):
        for ci in range(n_chunks):
            v0 = ci * ch
            sz = min(ch, NV - v0)
            da = loopp.tile([128, 3, ch], F16, tag="da")
            nc.sync.dma_start(da[:, :, 0:sz], dirs_ap[0:128, :, v0:v0 + sz])
            db = loopp.tile([90, 3, ch], F16, tag="db")
            nc.sync.dma_start(db[:, :, 0:sz], dirs_ap[128:KC, :, v0:v0 + sz])

            vp_sb = loopp.tile([B_LOC, 3, ch], F16, tag="vp")
            for c in range(3):
                pvc = psMM.tile([B_LOC, ch], F32, tag="mm")
                nc.tensor.matmul(pvc[:, 0:sz], coeffT_a[:, :], da[:, c, 0:sz],
                                 start=True, stop=False)
                nc.tensor.matmul(pvc[:, 0:sz], coeffT_b[:, :], db[:, c, 0:sz],
                                 start=False, stop=True)
                S.copy(vp_sb[:, c, 0:sz], pvc[:, 0:sz])

            t_sbs = []
            for n in range(3):
                ptn = psTG.tile([B_LOC, 3, ch], F32, tag="tg")
                for m in range(3):
                    e = m * 4 + n
                    nc.tensor.matmul(ptn[:, m, 0:sz],
                                     gat[:, e * B_LOC:(e + 1) * B_LOC],
                                     wt_sb[:, v0:v0 + sz], start=True, stop=True)
                t_sb = loopp.tile([B_LOC, 3, ch], F16, tag=f"tsb{n}")
                if sz == ch:
                    S.copy(t_sb[:, :, :], ptn[:, :, :])
                else:
                    for m in range(3):
                        S.copy(t_sb[:, m, 0:sz], ptn[:, m, 0:sz])
                t_sbs.append(t_sb)

            pt3 = psTG.tile([B_LOC, 3, ch], F32, tag="tg")
            for m in range(3):
                e = m * 4 + 3
                nc.tensor.matmul(pt3[:, m, 0:sz],
                                 gat[:, e * B_LOC:(e + 1) * B_LOC],
                                 wt_sb[:, v0:v0 + sz], start=True, stop=True)

            tmps = []
            for n in range(3):
                tmp = loopp.tile([B_LOC, 3, ch], F16, tag=f"tmp{n}")
                vb = vp_sb[:, n, 0:sz].unsqueeze(1).broadcast_to([B_LOC, 3, sz])
                eng = P if n == 2 else V
                eng.tensor_mul(tmp[:, :, 0:sz], t_sbs[n][:, :, 0:sz], vb)
                tmps.append(tmp)
            acc = loopp.tile([B_LOC, 3, ch], F16, tag="acc")
            V.tensor_add(acc[:, :, 0:sz], tmps[0][:, :, 0:sz], pt3[:, :, 0:sz])
            V.tensor_add(acc[:, :, 0:sz], acc[:, :, 0:sz], tmps[1][:, :, 0:sz])
            out_sb = loopp.tile([B_LOC, 3, ch], ODT, tag="outsb")
            V.tensor_add(out_sb[:, :, 0:sz], acc[:, :, 0:sz], tmps[2][:, :, 0:sz])
            nc.sync.dma_start(out_d.ap()[:, :, v0:v0 + sz], out_sb[:, :, 0:sz])


def _host_prep(inputs, cfg):
    fp16 = cfg["compute"] == "fp16"
    cdt = np.float16 if fp16 else np.float32
    shapedirs = np.asarray(inputs["shapedirs"], np.float32)
    posedirs = np.asarray(inputs["posedirs"], np.float32)
    v_template = np.asarray(inputs["v_template"], np.float32)
    Jreg = np.asarray(inputs["J_regressor"], np.float32)
    weights = np.asarray(inputs["weights"], np.float32)

    dirs = np.empty((KC, VC), np.float32)
    dirs[0:10] = shapedirs.transpose(2, 1, 0).reshape(10, VC)
    dirs[10:217] = posedirs.transpose(2, 1, 0).reshape(NP, VC)
    dirs[217] = v_template.T.reshape(VC)
    js2 = np.einsum('jv,vcs->sjc', Jreg, shapedirs).reshape(10, 72)
    jtmpl = (Jreg @ v_template).reshape(1, 72)
    return {
        "dirs": np.ascontiguousarray(dirs.astype(cdt)),
        "wt": np.ascontiguousarray(weights.T.astype(cdt)),
        "js2": np.ascontiguousarray(js2),
        "jtmpl": np.ascontiguousarray(jtmpl),
        "ident": np.eye(128, dtype=np.float32),
    }
